# revision 1
# baseline (speedup 1.0000x reference)
"""MultiHeadAttention Trainium2 Bass kernel (8-core SPMD).

Problem: B=2, S=2048, DIM=1024, H=16 heads (dh=64), fp32 reference.
Sharding: core c handles batch b = c//4 and 4 heads ho = 4*(c%4)..+4
(data-parallel over batch x tensor-parallel over heads). Each core:
  qhT/khT = W{q,k}.T-slice @ x.T + b   -> [256, 2048] bf16 (head-dim major)
  vh_aug  = x @ Wv.T-slice + bv (+ones col per head) per k-tile
  scores^T = kh @ qh.T (per head, K=64 row-packed pairs)
  P^T = keepmask * exp(SCALE * scores^T)                (ACT + DVE)
  out^T[65|128, q] = [vh | ones].T @ P^T                (PV + row-sums fused)
  normalize by 1/sums (per-qt batched reciprocal), y^T = Wo.T-slice.T @ O^T
Host gathers: y[b] = sum over 4 cores of y^T_partial.T, + bo.

All bulk tensors are pre-tiled on the host into the exact [128, 512]/[128,
1024] tiles the kernel consumes, so every DMA is one fully contiguous
128/256KB transfer (strided 1KB-row DMAs measured at only ~190GB/s
aggregate -- packet-rate-bound).
"""

import os
import sys

sys.path.insert(0, "/opt/trn_rl_repo")
os.environ.setdefault("MYCRO_LOCAL_CACHE", "1")

import numpy as np

import concourse.bass as bass
import concourse.bacc as bacc
import concourse.tile as tile
from concourse import mybir
from concourse import bass_utils

F32 = mybir.dt.float32
BF16 = mybir.dt.bfloat16
NP_BF16 = mybir.dt.np(BF16)

B, S, DIM = 2, 2048, 1024
H = 16
DH = 64
SCALE = 1.0 / (DIM ** 0.5)
N_CORES = 8
HPC = 4          # heads per core
QT = S // 512    # 4 q-chunks of 512
KT = S // 128    # 16 k-tiles of 128
CT = DIM // 128  # 8 contraction tiles for projections

# vh_aug per-kt layout: per pair p (2 local pairs):
#   A block: [vh_A(64) | ones(1)]                 at cols p*193 + [0, 65)
#   B block: [zeros(32) | ones(1) | zeros(31) | vh_B(64)] at cols p*193 + [65, 193)
#   (B ones at col +97 so B sums land on psum partition 32 -- DVE start
#   partitions must be in {0, 32, 64, 96})
VHA_W = 386


def build_nc():
    # Bacc (not plain Bass): its compile() pipeline splits multi-semaphore
    # waits into event-semaphore chains -- walrus codegen allows only ONE
    # sync wait per compute instruction on TRN2.
    nc = bacc.Bacc("TRN2", target_bir_lowering=False)

    xq_d = nc.declare_dram_parameter("xq", [QT, CT, 128, 512], BF16, isOutput=False)
    xk_d = nc.declare_dram_parameter("xk", [QT, CT, 128, 512], BF16, isOutput=False)
    xv_d = nc.declare_dram_parameter("xv", [QT, CT, 128, 512], BF16, isOutput=False)
    wq_d = nc.declare_dram_parameter("wq", [CT, 128, 256], BF16, isOutput=False)
    wk_d = nc.declare_dram_parameter("wk", [CT, 128, 256], BF16, isOutput=False)
    wv_d = nc.declare_dram_parameter("wv", [CT, 128, 256], BF16, isOutput=False)
    wo_d = nc.declare_dram_parameter("wo", [2, 128, 1024], BF16, isOutput=False)
    bq_d = nc.declare_dram_parameter("bq2", [2, 128, 1], F32, isOutput=False)
    bk_d = nc.declare_dram_parameter("bk2", [2, 128, 1], F32, isOutput=False)
    bvb_d = nc.declare_dram_parameter("bvb", [128, 256], BF16, isOutput=False)
    mk_d = nc.declare_dram_parameter("mk", [KT, QT, 128, 512], BF16, isOutput=False)
    yt_d = nc.declare_dram_parameter("yt", [8, 2, 128, 1024], BF16, isOutput=True)
    rscr_d = nc.dram_tensor("rscr", [HPC, S], F32)

    with tile.TileContext(nc) as tc:
        with tc.tile_pool(name="persist", bufs=1) as singles:
            # ---- biases + weights FIRST on the DMA queues (a late bias
            # gates the first DVE cast and stalls the whole machine) ----
            bq_sb, bk_sb = [], []
            for m in range(2):
                tq = singles.tile([128, 1], F32, tag=f"bq{m}", name=f"bq{m}")
                nc.sync.dma_start(out=tq, in_=bq_d[m])
                bq_sb.append(tq)
                tk = singles.tile([128, 1], F32, tag=f"bk{m}", name=f"bk{m}")
                nc.sync.dma_start(out=tk, in_=bk_d[m])
                bk_sb.append(tk)
            bvb_sb = singles.tile([128, 256], BF16, tag="bvb")
            nc.sync.dma_start(out=bvb_sb, in_=bvb_d[:, :])

            def load_rows(dram, n_tiles, width, tag):
                tiles = []
                for c in range(n_tiles):
                    t = singles.tile([128, width], BF16, tag=f"{tag}{c}", name=f"{tag}{c}")
                    nc.sync.dma_start(out=t, in_=dram[c])
                    tiles.append(t)
                return tiles

            wq_sb = load_rows(wq_d, CT, 256, "wq")
            wk_sb = load_rows(wk_d, CT, 256, "wk")
            wv_sb = load_rows(wv_d, CT, 256, "wv")
            wo_sb = load_rows(wo_d, 2, DIM, "wo")

            def load_x(dram, tag):
                # chunk-major: all 8 c-tiles of a column chunk arrive together
                tiles = [[None] * QT for _ in range(CT)]
                for n in range(QT):
                    for c in range(CT):
                        t = singles.tile([128, 512], BF16,
                                         tag=f"{tag}{c}_{n}", name=f"{tag}{c}_{n}")
                        nc.sync.dma_start(out=t, in_=dram[n, c])
                        tiles[c][n] = t
                return tiles

            xv_sb = load_x(xv_d, "xv")
            xk_sb = load_x(xk_d, "xk")
            xq_sb = load_x(xq_d, "xq")

            # ---- persistent intermediates ----
            qhT = [[singles.tile([128, 512], BF16, tag=f"qhT{m}_{n}",
                                 name=f"qhT{m}_{n}") for n in range(QT)]
                   for m in range(2)]
            khT = [[singles.tile([128, 512], BF16, tag=f"khT{m}_{n}",
                                 name=f"khT{m}_{n}") for n in range(QT)]
                   for m in range(2)]
            OT = [singles.tile([128, S], BF16, tag=f"OT{m}", name=f"OT{m}") for m in range(2)]
            vha = [singles.tile([128, VHA_W], BF16, tag=f"vha{kt}",
                                name=f"vha{kt}") for kt in range(KT)]
            sums_stage = singles.tile([128, 2, S], F32, tag="sums_stage")

            for kt in range(KT):
                for p in range(2):
                    base = p * 193
                    nc.gpsimd.memset(vha[kt][:, base + 64:base + 65], 1.0)
                    nc.gpsimd.memset(vha[kt][:, base + 97:base + 98], 1.0)
                    nc.gpsimd.memset(vha[kt][:, base + 65:base + 97], 0.0)
                    nc.gpsimd.memset(vha[kt][:, base + 98:base + 129], 0.0)

            # ---- projections (own scoped psum pool, v1-style) ----
            with tc.tile_pool(name="pjp", bufs=2, space="PSUM") as pj:
                # PE warmup to open the HAM clock gate while DMAs land
                warm = singles.tile([128, 512], BF16, tag="warm")
                nc.gpsimd.memset(warm[:, :], 0.0)
                wps = pj.tile([128, 512], F32, tag="pqk", name="wps")
                for i in range(24):
                    nc.tensor.matmul(
                        wps, warm[:, 0:128], warm[:, :],
                        start=True, stop=True)

                for kt in range(KT):
                    ps = pj.tile([128, 256], F32, tag="pv", name="psv")
                    for c in range(CT):
                        nc.tensor.matmul(
                            ps,
                            xv_sb[c][kt // 4][:, (kt % 4) * 128:(kt % 4 + 1) * 128],
                            wv_sb[c],
                            start=(c == 0),
                            stop=(c == CT - 1),
                        )
                    for h in range(HPC):
                        p, is_b = h // 2, h % 2
                        col = p * 193 + (129 if is_b else 0)
                        nc.vector.tensor_tensor(
                            out=vha[kt][:, col:col + 64],
                            in0=ps[:, h * 64:(h + 1) * 64],
                            in1=bvb_sb[:, h * 64:(h + 1) * 64],
                            op=mybir.AluOpType.add,
                        )
                for x_sb, w_sb, b_sb, dst in (
                    (xk_sb, wk_sb, bk_sb, khT),
                    (xq_sb, wq_sb, bq_sb, qhT),
                ):
                    for m in range(2):
                        for n in range(QT):
                            ps = pj.tile([128, 512], F32, tag="pqk", name="psqk")
                            for c in range(CT):
                                nc.tensor.matmul(
                                    ps,
                                    w_sb[c][:, m * 128:(m + 1) * 128],
                                    x_sb[c][n],
                                    start=(c == 0),
                                    stop=(c == CT - 1),
                                )
                            bb = b_sb[m][:, 0:1]
                            bb_bc = bass.AP(
                                tensor=bb.tensor, offset=bb.offset,
                                ap=[list(bb.ap[0]), [0, 512]])
                            nc.vector.tensor_tensor(
                                out=dst[m][n],
                                in0=ps,
                                in1=bb_bc,
                                op=mybir.AluOpType.add,
                            )

            # ---- attention: v1 structure (pair-merged, shared mask) ----
            with tc.tile_pool(name="scp", bufs=2, space="PSUM") as scp, \
                 tc.tile_pool(name="pvp", bufs=2, space="PSUM") as pvp:
                for qt in range(QT):
                    po = [pvp.tile([128, 1024], F32, tag="po", name="po")
                          for _ in range(2)]
                    for kt in range(KT):
                        mt = singles.tile([128, 512], BF16, tag="mask",
                                          name="mask", bufs=6)
                        nc.sync.dma_start(out=mt, in_=mk_d[kt, qt])
                        m_ap = mt[:, :]
                        mbc = bass.AP(
                            tensor=m_ap.tensor,
                            offset=m_ap.offset,
                            ap=[list(m_ap.ap[0]), [0, 2], list(m_ap.ap[1])],
                        )
                        for p in range(2):
                            ps = scp.tile([128, 1024], F32, tag="sc", name="ps")
                            for ab in range(2):
                                nc.tensor.matmul(
                                    ps[:, ab * 512:(ab + 1) * 512],
                                    khT[p][kt // 4][ab * 64:(ab + 1) * 64,
                                                    (kt % 4) * 128:(kt % 4 + 1) * 128],
                                    qhT[p][qt][ab * 64:(ab + 1) * 64, :],
                                    start=True,
                                    stop=True,
                                )
                            pt = singles.tile([128, 1024], BF16, tag="pt",
                                              name="pt", bufs=4)
                            nc.scalar.activation(
                                out=pt, in_=ps,
                                func=mybir.ActivationFunctionType.Exp,
                                scale=float(SCALE),
                            )
                            nc.vector.tensor_tensor(
                                out=pt, in0=pt, in1=mbc,
                                op=mybir.AluOpType.mult,
                            )
                            base = p * 193
                            nc.tensor.matmul(
                                po[p][0:65, 0:512],
                                vha[kt][:, base:base + 65],
                                pt[:, 0:512],
                                start=(kt == 0), stop=(kt == KT - 1),
                            )
                            nc.tensor.matmul(
                                po[p][:, 512:1024],
                                vha[kt][:, base + 65:base + 193],
                                pt[:, 512:1024],
                                start=(kt == 0), stop=(kt == KT - 1),
                            )
                    for p in range(2):
                        qsl = slice(qt * 512, (qt + 1) * 512)
                        nc.vector.tensor_copy(
                            out=OT[p][0:64, qsl], in_=po[p][0:64, 0:512])
                        nc.vector.tensor_copy(
                            out=OT[p][64:128, qsl], in_=po[p][64:128, 512:1024])
                        nc.vector.tensor_copy(
                            out=sums_stage[64:65, p, qsl],
                            in_=po[p][64:65, 0:512])
                        nc.vector.tensor_copy(
                            out=sums_stage[32:33, p, qsl],
                            in_=po[p][32:33, 512:1024])

                # ---- batched normalization ----
                recin = singles.tile([128, 64], F32, tag="recin")
                for h in range(HPC):
                    p, is_b = h // 2, h % 2
                    row = 32 if is_b else 64
                    nc.sync.dma_start(
                        out=recin[:, h * 16:(h + 1) * 16],
                        in_=sums_stage[row:row + 1, p, :])
                recout = singles.tile([128, 64], F32, tag="recout")
                nc.vector.reciprocal(out=recout, in_=recin)
                for h in range(HPC):
                    nc.sync.dma_start(
                        out=rscr_d[h:h + 1, :],
                        in_=recout[:, h * 16:(h + 1) * 16])
                for p in range(2):
                    rbc = singles.tile([128, S], F32, tag=f"rbc{p}", name=f"rbc{p}")
                    for ab in range(2):
                        srow = rscr_d[2 * p + ab:2 * p + ab + 1, :]
                        src_bc = bass.AP(
                            tensor=srow.tensor,
                            offset=srow.offset,
                            ap=[[0, 64], list(srow.ap[-1])],
                        )
                        nc.sync.dma_start(
                            out=rbc[ab * 64:(ab + 1) * 64, :], in_=src_bc)
                    nc.vector.tensor_tensor(
                        out=OT[p], in0=OT[p], in1=rbc,
                        op=mybir.AluOpType.mult)

            # ---- output projection ----
            with tc.tile_pool(name="oyp", bufs=4, space="PSUM") as oyp:
                for ot in range(8):
                    for half in range(2):
                        ps = oyp.tile([128, 1024], F32, tag="py", name="psy")
                        for p in range(2):
                            for n in range(2):
                                nc.tensor.matmul(
                                    ps[:, n * 512:(n + 1) * 512],
                                    wo_sb[p][:, ot * 128:(ot + 1) * 128],
                                    OT[p][:, (half * 2 + n) * 512:
                                          (half * 2 + n + 1) * 512],
                                    start=(p == 0),
                                    stop=(p == 1),
                                )
                        yt = singles.tile([128, 1024], BF16, tag="yt",
                                          name="yt", bufs=4)
                        nc.scalar.copy(out=yt, in_=ps)
                        nc.sync.dma_start(out=yt_d[ot, half], in_=yt)
    nc.compile()
    return nc


_NC_CACHE = None


def get_nc():
    global _NC_CACHE
    if _NC_CACHE is None:
        _NC_CACHE = build_nc()
    return _NC_CACHE


def _tile_x(xT):
    # [1024, 2048] -> [QT, CT, 128, 512]
    return np.ascontiguousarray(
        xT.reshape(CT, 128, QT, 512).transpose(2, 0, 1, 3))


def prep_in_maps(q, k, v, mask, Wq, bq, Wk, bk, Wv, bv, Wo, bo):
    q = np.asarray(q, np.float32)
    k = np.asarray(k, np.float32)
    v = np.asarray(v, np.float32)
    mask = np.asarray(mask)
    WqT = np.asarray(Wq, np.float32).T
    WkT = np.asarray(Wk, np.float32).T
    WvT = np.asarray(Wv, np.float32).T
    WoT = np.asarray(Wo, np.float32).T
    bq = np.asarray(bq, np.float32)
    bk = np.asarray(bk, np.float32)
    bv = np.asarray(bv, np.float32)

    xT = {}
    keepT = {}
    for b in range(B):
        xT[b] = (
            _tile_x(np.ascontiguousarray(q[b].T).astype(NP_BF16)),
            _tile_x(np.ascontiguousarray(k[b].T).astype(NP_BF16)),
            _tile_x(np.ascontiguousarray(v[b].T).astype(NP_BF16)),
        )
        mt = np.ascontiguousarray((~mask[b, 0]).T.astype(np.float32)).astype(NP_BF16)
        keepT[b] = np.ascontiguousarray(
            mt.reshape(KT, 128, QT, 512).transpose(0, 2, 1, 3))

    in_maps = []
    for c in range(N_CORES):
        b = c // 4
        ho = c % 4
        dsl = slice(ho * 256, ho * 256 + 256)
        xq, xk, xv = xT[b]
        in_maps.append({
            "xq": xq,
            "xk": xk,
            "xv": xv,
            "wq": np.ascontiguousarray(WqT[:, dsl]).astype(NP_BF16).reshape(CT, 128, 256),
            "wk": np.ascontiguousarray(WkT[:, dsl]).astype(NP_BF16).reshape(CT, 128, 256),
            "wv": np.ascontiguousarray(WvT[:, dsl]).astype(NP_BF16).reshape(CT, 128, 256),
            "wo": np.ascontiguousarray(WoT[dsl, :]).astype(NP_BF16).reshape(2, 128, 1024),
            "bq2": np.ascontiguousarray(bq[dsl]).reshape(2, 128, 1).astype(np.float32),
            "bk2": np.ascontiguousarray(bk[dsl]).reshape(2, 128, 1).astype(np.float32),
            "bvb": np.ascontiguousarray(
                np.broadcast_to(bv[dsl], (128, 256))).astype(NP_BF16),
            "mk": keepT[b],
        })
    return in_maps


def gather_output(results, bo):
    bo = np.asarray(bo, np.float32)
    y = np.zeros((B, S, DIM), np.float32)
    for c in range(N_CORES):
        yt = np.asarray(results[c]["yt"], np.float32)  # [8, 2, 128, 1024]
        yT = yt.transpose(0, 2, 1, 3).reshape(DIM, S)
        y[c // 4] += yT.T
    y += bo[None, None, :]
    return y


def kernel(**inputs):
    nc = get_nc()
    in_maps = prep_in_maps(**{k_: inputs[k_] for k_ in (
        "q", "k", "v", "mask", "Wq", "bq", "Wk", "bk", "Wv", "bv", "Wo", "bo")})
    res = bass_utils.run_bass_kernel_spmd(nc, in_maps, list(range(N_CORES)))
    return gather_output(res.results, inputs["bo"])



# revision 11
# speedup vs baseline: 1.0770x; 1.0770x over previous
"""MultiHeadAttention Trainium2 Bass kernel (8-core SPMD), v2.

Problem: B=2, S=2048, DIM=1024, H=16 heads (dh=64), fp32 reference.
Sharding: core c handles batch b = c//4 and 4 heads ho = 4*(c%4)..+4.

v2 changes vs v1 (332us -> target ~180us):
- q/k path in fp8e4m3 (x AND W, host-validated rel-err 3.6e-3 vs 2e-2
  budget); v path stays bf16 (fp8 there costs 1.3e-2).
- q/k projections use MatmulPerfMode.DoubleRow (fp8 K=256 contraction).
- x / W tensors shipped as single [128, CT, *] tiles -> one large DMA
  each with 4-32KB per-partition contiguous runs (~350+ GB/s vs ~200).
- Per-qt pipelined softmax normalization: sums -> recip -> selector-
  matmul partition-broadcast -> in-place OT scale, all hidden under the
  next qt's attention (replaces a 20us end-of-kernel stall + DRAM
  round-trips).
- Output projection PSUM reuses the scores pool rotation so the PE
  never idles between attention and oproj (HAM stays warm); PSUM->SBUF
  output copies alternate ACT/DVE.
- Attention phase is ACT(exp)-bound (~1us per [128,1024] exp); PE/DVE
  loads are kept strictly below that (~0.86us / ~0.85us per half-tile).
"""

import os
import sys

sys.path.insert(0, "/opt/trn_rl_repo")
os.environ.setdefault("MYCRO_LOCAL_CACHE", "1")

import numpy as np

import concourse.bass as bass
import concourse.bacc as bacc
import concourse.tile as tile
from concourse import mybir
from concourse import bass_utils

F32 = mybir.dt.float32
BF16 = mybir.dt.bfloat16
F8 = mybir.dt.float8e4
NP_BF16 = mybir.dt.np(BF16)
NP_F8 = mybir.dt.np(F8)
DR = mybir.MatmulPerfMode.DoubleRow

B, S, DIM = 2, 2048, 1024
H = 16
DH = 64
SCALE = 1.0 / (DIM ** 0.5)
N_CORES = 8
HPC = 4          # heads per core
QT = S // 512    # 4 q-chunks of 512
KT = S // 128    # 16 k-tiles of 128
CT = DIM // 128  # 8 contraction tiles for projections

# vh_aug per-kt layout (unchanged from v1): per pair p (2 local pairs):
#   A block: [vh_A(64) | ones(1)]                 at cols p*193 + [0, 65)
#   B block: [zeros(32) | ones(1) | zeros(31) | vh_B(64)] at cols p*193 + [65, 193)
VHA_W = 386


def build_nc():
    nc = bacc.Bacc("TRN2", target_bir_lowering=False)

    xq_d = nc.declare_dram_parameter("xq", [128, CT, S], F8, isOutput=False)
    xk_d = nc.declare_dram_parameter("xk", [128, CT, S], F8, isOutput=False)
    xv_d = nc.declare_dram_parameter("xv", [128, CT, S], BF16, isOutput=False)
    wq_d = nc.declare_dram_parameter("wq", [128, CT, 256], F8, isOutput=False)
    wk_d = nc.declare_dram_parameter("wk", [128, CT, 256], F8, isOutput=False)
    wv_d = nc.declare_dram_parameter("wv", [128, CT, 256], BF16, isOutput=False)
    wo_d = nc.declare_dram_parameter("wo", [2, 128, 1024], BF16, isOutput=False)
    bq_d = nc.declare_dram_parameter("bq2", [2, 128, 1], F32, isOutput=False)
    bk_d = nc.declare_dram_parameter("bk2", [2, 128, 1], F32, isOutput=False)
    bvb_d = nc.declare_dram_parameter("bvb", [128, 256], BF16, isOutput=False)
    mk_d = nc.declare_dram_parameter("mk", [KT, QT, 128, 512], BF16, isOutput=False)
    sel_d = nc.declare_dram_parameter("selc", [2, 128], BF16, isOutput=False)
    yt_d = nc.declare_dram_parameter("yt", [8, 128, 2048], BF16, isOutput=True)

    with tile.TileContext(nc) as tc:
        with tc.tile_pool(name="persist", bufs=1) as singles:
            # ---- small operands first on the DMA queue ----
            bq_sb, bk_sb = [], []
            for m in range(2):
                tq = singles.tile([128, 1], F32, tag=f"bq{m}", name=f"bq{m}")
                nc.sync.dma_start(out=tq, in_=bq_d[m])
                bq_sb.append(tq)
                tk = singles.tile([128, 1], F32, tag=f"bk{m}", name=f"bk{m}")
                nc.sync.dma_start(out=tk, in_=bk_d[m])
                bk_sb.append(tk)
            bvb_sb = singles.tile([128, 256], BF16, tag="bvb")
            nc.sync.dma_start(out=bvb_sb, in_=bvb_d[:, :])

            wk_sb = singles.tile([128, CT, 256], F8, tag="wk", name="wk")
            nc.sync.dma_start(out=wk_sb, in_=wk_d[:, :, :])
            wv_sb = singles.tile([128, CT, 256], BF16, tag="wv", name="wv")
            nc.sync.dma_start(out=wv_sb, in_=wv_d[:, :, :])
            wq_sb = singles.tile([128, CT, 256], F8, tag="wq", name="wq")
            nc.sync.dma_start(out=wq_sb, in_=wq_d[:, :, :])
            wo_sb = []
            for m in range(2):
                t = singles.tile([128, 1024], BF16, tag=f"wo{m}", name=f"wo{m}")
                nc.sync.dma_start(out=t, in_=wo_d[m])
                wo_sb.append(t)

            # ---- bulk x, in consumption order: k, v, q ----
            xk_sb = singles.tile([128, CT, S], F8, tag="xk", name="xk")
            nc.sync.dma_start(out=xk_sb, in_=xk_d[:, :, :])
            xv_sb = singles.tile([128, CT, S], BF16, tag="xv", name="xv")
            nc.sync.dma_start(out=xv_sb, in_=xv_d[:, :, :])
            xq_sb = singles.tile([128, CT, S], F8, tag="xq", name="xq")
            nc.sync.dma_start(out=xq_sb, in_=xq_d[:, :, :])

            # ---- persistent intermediates ----
            qhT = [singles.tile([128, S], BF16, tag=f"qhT{m}", name=f"qhT{m}")
                   for m in range(2)]
            khT = [singles.tile([128, S], BF16, tag=f"khT{m}", name=f"khT{m}")
                   for m in range(2)]
            OT = [singles.tile([128, S], BF16, tag=f"OT{m}", name=f"OT{m}")
                  for m in range(2)]
            vha = [singles.tile([128, VHA_W], BF16, tag=f"vha{kt}",
                                name=f"vha{kt}") for kt in range(KT)]
            sums_st = singles.tile([128, 2, 512], F32, tag="sums_st")
            sel_sb = singles.tile([2, 128], BF16, tag="sel")

            for kt in range(KT):
                for p in range(2):
                    base = p * 193
                    nc.gpsimd.memset(vha[kt][:, base + 64:base + 65], 1.0)
                    nc.gpsimd.memset(vha[kt][:, base + 97:base + 98], 1.0)
                    nc.gpsimd.memset(vha[kt][:, base + 65:base + 97], 0.0)
                    nc.gpsimd.memset(vha[kt][:, base + 98:base + 129], 0.0)
            # selector: row0 (gathered from partition 32) = B sums -> dims 64:128
            #           row1 (partition 64) = A sums -> dims 0:64
            nc.sync.dma_start(out=sel_sb, in_=sel_d[:, :])
            nc.gpsimd.memset(sums_st[:, :, :], 1.0)

            # ---- projections ----
            with tc.tile_pool(name="pjp", bufs=2, space="PSUM") as pj:
                # PE warmup to open the HAM clock gate while DMAs land
                warm = singles.tile([128, 512], BF16, tag="warm")
                nc.gpsimd.memset(warm[:, :], 0.0)
                wps = pj.tile([128, 512], F32, tag="pwarm", name="wps")
                for i in range(24):
                    nc.tensor.matmul(
                        wps, warm[:, 0:128], warm[:, :],
                        start=True, stop=True)

                def qk_proj(w_sb, x_sb, b_sb, dst, m):
                    # weight (c-pair, m-half) stays stationary across the 4
                    # n-chunks -> DoubleRow LDWEIGHTS amortized 4x.
                    pss = [pj.tile([128, 512], F32, tag=f"pqk{n}",
                                   name=f"psqk{n}", bufs=1) for n in range(4)]
                    for ci in range(4):
                        for n in range(4):
                            nc.tensor.matmul(
                                pss[n],
                                w_sb[:, 2 * ci:2 * ci + 2, m * 128:(m + 1) * 128],
                                x_sb[:, 2 * ci:2 * ci + 2, n * 512:(n + 1) * 512],
                                start=(ci == 0),
                                stop=(ci == 3),
                                perf_mode=DR,
                            )
                    bb = b_sb[m][:, 0:1]
                    bb_bc = bass.AP(
                        tensor=bb.tensor, offset=bb.offset,
                        ap=[list(bb.ap[0]), [0, 512]])
                    for n in range(4):
                        nc.vector.tensor_tensor(
                            out=dst[m][:, n * 512:(n + 1) * 512],
                            in0=pss[n], in1=bb_bc,
                            op=mybir.AluOpType.add,
                        )

                qk_proj(wk_sb, xk_sb, bk_sb, khT, 0)
                # v-projection (bf16)
                for kt in range(KT):
                    ps = pj.tile([128, 256], F32, tag="pv", name="psv")
                    for c in range(CT):
                        nc.tensor.matmul(
                            ps,
                            xv_sb[:, c, kt * 128:(kt + 1) * 128],
                            wv_sb[:, c, :],
                            start=(c == 0),
                            stop=(c == CT - 1),
                        )
                    for h in range(HPC):
                        p, is_b = h // 2, h % 2
                        col = p * 193 + (129 if is_b else 0)
                        nc.vector.tensor_tensor(
                            out=vha[kt][:, col:col + 64],
                            in0=ps[:, h * 64:(h + 1) * 64],
                            in1=bvb_sb[:, h * 64:(h + 1) * 64],
                            op=mybir.AluOpType.add,
                        )
                qk_proj(wk_sb, xk_sb, bk_sb, khT, 1)
                qk_proj(wq_sb, xq_sb, bq_sb, qhT, 0)
                qk_proj(wq_sb, xq_sb, bq_sb, qhT, 1)
                # (m is the head-pair index; both m needed before attention)

            # ---- attention + pipelined normalization + oproj ----
            with tc.tile_pool(name="scp", bufs=2, space="PSUM") as scp, \
                 tc.tile_pool(name="pvp", bufs=2, space="PSUM") as pvp:

                def emit_norm_head(qt, po):
                    """Steps at qt end: stage sums + free po via plain copies."""
                    qsl = slice(qt * 512, (qt + 1) * 512)
                    for p in range(2):
                        nc.vector.tensor_copy(
                            out=sums_st[64:65, p, :], in_=po[p][64:65, 0:512])
                        nc.vector.tensor_copy(
                            out=sums_st[32:33, p, :], in_=po[p][32:33, 512:1024])
                    for p in range(2):
                        nc.vector.tensor_copy(
                            out=OT[p][0:64, qsl], in_=po[p][0:64, 0:512])
                        nc.vector.tensor_copy(
                            out=OT[p][64:128, qsl], in_=po[p][64:128, 512:1024])

                def make_norm_tail(qt):
                    """Deferred steps, interleaved into the next qt."""
                    qsl = slice(qt * 512, (qt + 1) * 512)
                    rec_in = singles.tile([2, 2, 512], F32, tag="rec_in",
                                          name="rec_in", bufs=2)
                    rec_f = singles.tile([2, 2, 512], F32, tag="rec_f",
                                         name="rec_f", bufs=2)
                    rec_bf = singles.tile([2, 2, 512], BF16, tag="rec_bf",
                                          name="rec_bf", bufs=2)
                    rbc = [None, None]

                    def s_gather():
                        s32 = sums_st[32:33, :, :]
                        s64 = sums_st[64:65, :, :]
                        src = bass.AP(
                            tensor=s32.tensor, offset=s32.offset,
                            ap=[[s64.offset - s32.offset, 2],
                                list(s32.ap[1]), list(s32.ap[2])])
                        nc.sync.dma_start(out=rec_in, in_=src)

                    def s_recip():
                        nc.vector.reciprocal(out=rec_f, in_=rec_in)
                        nc.vector.tensor_copy(out=rec_bf, in_=rec_f)

                    def s_bcast(p):
                        def go():
                            rps = scp.tile([128, 1024], F32, tag="sc", name="rps")
                            nc.tensor.matmul(
                                rps[:, 0:512], sel_sb, rec_bf[:, p, :],
                                start=True, stop=True)
                            rb = singles.tile([128, 512], BF16, tag=f"rbc{p}",
                                              name=f"rbc{p}", bufs=2)
                            nc.vector.tensor_copy(out=rb, in_=rps[:, 0:512])
                            rbc[p] = rb
                        return go

                    def s_scale(p, is_b):
                        def go():
                            rows = slice(64, 128) if is_b else slice(0, 64)
                            nc.vector.tensor_tensor(
                                out=OT[p][rows, qsl], in0=OT[p][rows, qsl],
                                in1=rbc[p][rows, :],
                                op=mybir.AluOpType.mult,
                            )
                        return go

                    return [s_gather, s_recip, s_bcast(0), s_bcast(1),
                            s_scale(0, 0), s_scale(0, 1),
                            s_scale(1, 0), s_scale(1, 1)]

                pending = []
                for qt in range(QT):
                    po = [pvp.tile([128, 1024], F32, tag="po", name="po",
                                   bufs=2) for _ in range(2)]
                    for kt in range(KT):
                        mt = singles.tile([128, 512], BF16, tag="mask",
                                          name="mask", bufs=6)
                        nc.sync.dma_start(out=mt, in_=mk_d[kt, qt])
                        m_ap = mt[:, :]
                        mbc = bass.AP(
                            tensor=m_ap.tensor,
                            offset=m_ap.offset,
                            ap=[list(m_ap.ap[0]), [0, 2], list(m_ap.ap[1])],
                        )
                        for p in range(2):
                            ps = scp.tile([128, 1024], F32, tag="sc", name="ps")
                            for ab in range(2):
                                nc.tensor.matmul(
                                    ps[:, ab * 512:(ab + 1) * 512],
                                    khT[p][ab * 64:(ab + 1) * 64,
                                           kt * 128:(kt + 1) * 128],
                                    qhT[p][ab * 64:(ab + 1) * 64,
                                           qt * 512:(qt + 1) * 512],
                                    start=True,
                                    stop=True,
                                )
                            pt = singles.tile([128, 1024], BF16, tag="pt",
                                              name="pt", bufs=6)
                            nc.scalar.activation(
                                out=pt, in_=ps,
                                func=mybir.ActivationFunctionType.Exp,
                                scale=float(SCALE),
                            )
                            nc.vector.tensor_tensor(
                                out=pt, in0=pt, in1=mbc,
                                op=mybir.AluOpType.mult,
                            )
                            base = p * 193
                            nc.tensor.matmul(
                                po[p][0:65, 0:512],
                                vha[kt][:, base:base + 65],
                                pt[:, 0:512],
                                start=(kt == 0), stop=(kt == KT - 1),
                            )
                            nc.tensor.matmul(
                                po[p][:, 512:1024],
                                vha[kt][:, base + 65:base + 193],
                                pt[:, 512:1024],
                                start=(kt == 0), stop=(kt == KT - 1),
                            )
                        if pending:
                            pending.pop(0)()
                    while pending:
                        pending.pop(0)()
                    emit_norm_head(qt, po)
                    pending = make_norm_tail(qt)
                # last qt: emit its tail inline (oproj half1 will wait on it)
                while pending:
                    pending.pop(0)()

                # ---- output projection (reuses scp rotation; PE stays hot) ----
                yt_done = []
                for half in range(2):
                    for ot in range(8):
                        ps = scp.tile([128, 1024], F32, tag="sc", name="psy")
                        for p in range(2):
                            for n in range(2):
                                nc.tensor.matmul(
                                    ps[:, n * 512:(n + 1) * 512],
                                    wo_sb[p][:, ot * 128:(ot + 1) * 128],
                                    OT[p][:, (half * 2 + n) * 512:
                                          (half * 2 + n + 1) * 512],
                                    start=(p == 0),
                                    stop=(p == 1),
                                )
                        yt = singles.tile([128, 1024], BF16, tag="yt",
                                          name="yt", bufs=4)
                        if (half * 8 + ot) % 2 == 0:
                            nc.scalar.copy(out=yt, in_=ps)
                        else:
                            nc.vector.tensor_copy(out=yt, in_=ps)
                        nc.sync.dma_start(
                            out=yt_d[ot][:, half * 1024:(half + 1) * 1024],
                            in_=yt)
    nc.compile()
    return nc


_NC_CACHE = None


def get_nc():
    global _NC_CACHE
    if _NC_CACHE is None:
        _NC_CACHE = build_nc()
    return _NC_CACHE


def _tile_ct(xT):
    # [1024, N] -> [128, CT, N]  (c-block-major partition layout)
    n = xT.shape[1]
    return np.ascontiguousarray(xT.reshape(CT, 128, n).transpose(1, 0, 2))


def prep_in_maps(q, k, v, mask, Wq, bq, Wk, bk, Wv, bv, Wo, bo):
    q = np.asarray(q, np.float32)
    k = np.asarray(k, np.float32)
    v = np.asarray(v, np.float32)
    mask = np.asarray(mask)
    WqT = np.asarray(Wq, np.float32).T
    WkT = np.asarray(Wk, np.float32).T
    WvT = np.asarray(Wv, np.float32).T
    WoT = np.asarray(Wo, np.float32).T
    bq = np.asarray(bq, np.float32)
    bk = np.asarray(bk, np.float32)
    bv = np.asarray(bv, np.float32)

    xT = {}
    keepT = {}
    for b in range(B):
        xT[b] = (
            _tile_ct(np.ascontiguousarray(q[b].T)).astype(NP_F8),
            _tile_ct(np.ascontiguousarray(k[b].T)).astype(NP_F8),
            _tile_ct(np.ascontiguousarray(v[b].T)).astype(NP_BF16),
        )
        mt = np.ascontiguousarray((~mask[b, 0]).T.astype(np.float32)).astype(NP_BF16)
        keepT[b] = np.ascontiguousarray(
            mt.reshape(KT, 128, QT, 512).transpose(0, 2, 1, 3))

    sel = np.zeros((2, 128), np.float32)
    sel[0, 64:128] = 1.0  # row0 <- partition-32 (B) sums -> dims 64:128
    sel[1, 0:64] = 1.0    # row1 <- partition-64 (A) sums -> dims 0:64
    sel = sel.astype(NP_BF16)

    in_maps = []
    for c in range(N_CORES):
        b = c // 4
        ho = c % 4
        dsl = slice(ho * 256, ho * 256 + 256)
        xq, xk, xv = xT[b]
        in_maps.append({
            "xq": xq,
            "xk": xk,
            "xv": xv,
            "wq": _tile_ct(np.ascontiguousarray(WqT[:, dsl])).astype(NP_F8),
            "wk": _tile_ct(np.ascontiguousarray(WkT[:, dsl])).astype(NP_F8),
            "wv": _tile_ct(np.ascontiguousarray(WvT[:, dsl])).astype(NP_BF16),
            "wo": np.ascontiguousarray(WoT[dsl, :]).astype(NP_BF16).reshape(2, 128, 1024),
            "bq2": np.ascontiguousarray(bq[dsl]).reshape(2, 128, 1).astype(np.float32),
            "bk2": np.ascontiguousarray(bk[dsl]).reshape(2, 128, 1).astype(np.float32),
            "bvb": np.ascontiguousarray(
                np.broadcast_to(bv[dsl], (128, 256))).astype(NP_BF16),
            "mk": keepT[b],
            "selc": sel,
        })
    return in_maps


def gather_output(results, bo):
    bo = np.asarray(bo, np.float32)
    y = np.zeros((B, S, DIM), np.float32)
    for c in range(N_CORES):
        yt = np.asarray(results[c]["yt"], np.float32)  # [8, 128, 2048]
        yT = yt.reshape(DIM, S)
        y[c // 4] += yT.T
    y += bo[None, None, :]
    return y


def kernel(**inputs):
    nc = get_nc()
    in_maps = prep_in_maps(**{k_: inputs[k_] for k_ in (
        "q", "k", "v", "mask", "Wq", "bq", "Wk", "bk", "Wv", "bv", "Wo", "bo")})
    res = bass_utils.run_bass_kernel_spmd(nc, in_maps, list(range(N_CORES)))
    return gather_output(res.results, inputs["bo"])


# revision 18
# speedup vs baseline: 1.1851x; 1.1004x over previous
"""MultiHeadAttention Trainium2 Bass kernel (8-core SPMD), v2.

Problem: B=2, S=2048, DIM=1024, H=16 heads (dh=64), fp32 reference.
Sharding: core c handles batch b = c//4 and 4 heads ho = 4*(c%4)..+4.

v2 changes vs v1 (332us -> target ~180us):
- q/k path in fp8e4m3 (x AND W, host-validated rel-err 3.6e-3 vs 2e-2
  budget); v path stays bf16 (fp8 there costs 1.3e-2).
- q/k projections use MatmulPerfMode.DoubleRow (fp8 K=256 contraction).
- x / W tensors shipped as single [128, CT, *] tiles -> one large DMA
  each with 4-32KB per-partition contiguous runs (~350+ GB/s vs ~200).
- Per-qt pipelined softmax normalization: sums -> recip -> selector-
  matmul partition-broadcast -> in-place OT scale, all hidden under the
  next qt's attention (replaces a 20us end-of-kernel stall + DRAM
  round-trips).
- Output projection PSUM reuses the scores pool rotation so the PE
  never idles between attention and oproj (HAM stays warm); PSUM->SBUF
  output copies alternate ACT/DVE.
- Attention phase is ACT(exp)-bound (~1us per [128,1024] exp); PE/DVE
  loads are kept strictly below that (~0.86us / ~0.85us per half-tile).
"""

import os
import sys

sys.path.insert(0, "/opt/trn_rl_repo")
os.environ.setdefault("MYCRO_LOCAL_CACHE", "1")

import numpy as np

import concourse.bass as bass
import concourse.bacc as bacc
import concourse.tile as tile
from concourse import mybir
from concourse import bass_utils

F32 = mybir.dt.float32
BF16 = mybir.dt.bfloat16
F8 = mybir.dt.float8e4
NP_BF16 = mybir.dt.np(BF16)
NP_F8 = mybir.dt.np(F8)
DR = mybir.MatmulPerfMode.DoubleRow

B, S, DIM = 2, 2048, 1024
H = 16
DH = 64
SCALE = 1.0 / (DIM ** 0.5)
N_CORES = 8
HPC = 4          # heads per core
QT = S // 512    # 4 q-chunks of 512
KT = S // 128    # 16 k-tiles of 128
CT = DIM // 128  # 8 contraction tiles for projections

# vh_aug per-kt layout (unchanged from v1): per pair p (2 local pairs):
#   A block: [vh_A(64) | ones(1)]                 at cols p*193 + [0, 65)
#   B block: [zeros(32) | ones(1) | zeros(31) | vh_B(64)] at cols p*193 + [65, 193)
VHA_W = 386


def build_nc():
    nc = bacc.Bacc("TRN2", target_bir_lowering=False)

    xq_d = nc.declare_dram_parameter("xq", [128, CT, S], F8, isOutput=False)
    xk_d = nc.declare_dram_parameter("xk", [128, CT, S], F8, isOutput=False)
    xv_d = nc.declare_dram_parameter("xv", [128, CT, S], BF16, isOutput=False)
    wq_d = nc.declare_dram_parameter("wq", [128, CT, 256], F8, isOutput=False)
    wk_d = nc.declare_dram_parameter("wk", [128, CT, 256], F8, isOutput=False)
    wv_d = nc.declare_dram_parameter("wv", [128, CT, 256], BF16, isOutput=False)
    wo_d = nc.declare_dram_parameter("wo", [2, 128, 1024], BF16, isOutput=False)
    bq_d = nc.declare_dram_parameter("bq2", [2, 128, 1], F32, isOutput=False)
    bk_d = nc.declare_dram_parameter("bk2", [2, 128, 1], F32, isOutput=False)
    bvb_d = nc.declare_dram_parameter("bvb", [128, 256], BF16, isOutput=False)
    mk_d = nc.declare_dram_parameter("mk", [KT, QT, 128, 512], BF16, isOutput=False)
    sel_d = nc.declare_dram_parameter("selc", [2, 128], BF16, isOutput=False)
    yt_d = nc.declare_dram_parameter("yt", [8, 128, 2048], BF16, isOutput=True)

    with tile.TileContext(nc) as tc:
        with tc.tile_pool(name="persist", bufs=1) as singles:
            # ---- small operands first on the DMA queue ----
            bq_sb, bk_sb = [], []
            for m in range(2):
                tq = singles.tile([128, 1], F32, tag=f"bq{m}", name=f"bq{m}")
                nc.sync.dma_start(out=tq, in_=bq_d[m])
                bq_sb.append(tq)
                tk = singles.tile([128, 1], F32, tag=f"bk{m}", name=f"bk{m}")
                nc.sync.dma_start(out=tk, in_=bk_d[m])
                bk_sb.append(tk)
            bvb_sb = singles.tile([128, 256], BF16, tag="bvb")
            nc.sync.dma_start(out=bvb_sb, in_=bvb_d[:, :])

            wk_sb = singles.tile([128, CT, 256], F8, tag="wk", name="wk")
            nc.sync.dma_start(out=wk_sb, in_=wk_d[:, :, :])
            wv_sb = singles.tile([128, CT, 256], BF16, tag="wv", name="wv")
            nc.sync.dma_start(out=wv_sb, in_=wv_d[:, :, :])
            wq_sb = singles.tile([128, CT, 256], F8, tag="wq", name="wq")
            nc.sync.dma_start(out=wq_sb, in_=wq_d[:, :, :])
            wo_sb = []
            for m in range(2):
                t = singles.tile([128, 1024], BF16, tag=f"wo{m}", name=f"wo{m}")
                nc.sync.dma_start(out=t, in_=wo_d[m])
                wo_sb.append(t)

            # ---- bulk x, in consumption order: k, v, q ----
            xk_sb = singles.tile([128, CT, S], F8, tag="xk", name="xk")
            nc.sync.dma_start(out=xk_sb, in_=xk_d[:, :, :])
            xv_sb = singles.tile([128, CT, S], BF16, tag="xv", name="xv")
            nc.sync.dma_start(out=xv_sb, in_=xv_d[:, :, :])
            xq_sb = singles.tile([128, CT, S], F8, tag="xq", name="xq")
            nc.sync.dma_start(out=xq_sb, in_=xq_d[:, :, :])

            # ---- persistent intermediates ----
            qhT = [singles.tile([128, S], BF16, tag=f"qhT{m}", name=f"qhT{m}")
                   for m in range(2)]
            khT = [singles.tile([128, S], BF16, tag=f"khT{m}", name=f"khT{m}")
                   for m in range(2)]
            OT = [singles.tile([128, S], BF16, tag=f"OT{m}", name=f"OT{m}")
                  for m in range(2)]
            vha = [singles.tile([128, VHA_W], BF16, tag=f"vha{kt}",
                                name=f"vha{kt}") for kt in range(KT)]
            sums_st = singles.tile([128, 2, 512], F32, tag="sums_st")
            sel_sb = singles.tile([2, 128], BF16, tag="sel")

            # warm tile memset FIRST: the PE warmup gates on it, and the
            # vha/sums memsets behind it cost ~15us of gpsimd time.
            warm = singles.tile([128, 512], BF16, tag="warm")
            nc.gpsimd.memset(warm[:, :], 0.0)
            nc.gpsimd.memset(sums_st[:, :, :], 1.0)
            for kt in range(KT):
                for p in range(2):
                    base = p * 193
                    nc.gpsimd.memset(vha[kt][:, base + 64:base + 65], 1.0)
                    nc.gpsimd.memset(vha[kt][:, base + 97:base + 98], 1.0)
                    nc.gpsimd.memset(vha[kt][:, base + 65:base + 97], 0.0)
                    nc.gpsimd.memset(vha[kt][:, base + 98:base + 129], 0.0)
            # selector: row0 (gathered from partition 32) = B sums -> dims 64:128
            #           row1 (partition 64) = A sums -> dims 0:64
            nc.sync.dma_start(out=sel_sb, in_=sel_d[:, :])

            # ---- projections ----
            with tc.tile_pool(name="pjp", bufs=2, space="PSUM") as pj:
                # PE warmup to open the HAM clock gate while DMAs land
                wps = pj.tile([128, 512], F32, tag="pwarm", name="wps")
                for i in range(24):
                    nc.tensor.matmul(
                        wps, warm[:, 0:128], warm[:, :],
                        start=True, stop=True)

                def qk_proj(w_sb, x_sb, b_sb, dst, m):
                    # weight (c-pair, m-half) stays stationary across the 4
                    # n-chunks -> DoubleRow LDWEIGHTS amortized 4x.
                    pss = [pj.tile([128, 512], F32, tag=f"pqk{n}",
                                   name=f"psqk{n}", bufs=1) for n in range(4)]
                    for ci in range(4):
                        for n in range(4):
                            nc.tensor.matmul(
                                pss[n],
                                w_sb[:, 2 * ci:2 * ci + 2, m * 128:(m + 1) * 128],
                                x_sb[:, 2 * ci:2 * ci + 2, n * 512:(n + 1) * 512],
                                start=(ci == 0),
                                stop=(ci == 3),
                                perf_mode=DR,
                            )
                    bb = b_sb[m][:, 0:1]
                    bb_bc = bass.AP(
                        tensor=bb.tensor, offset=bb.offset,
                        ap=[list(bb.ap[0]), [0, 512]])
                    for n in range(4):
                        nc.vector.tensor_tensor(
                            out=dst[m][:, n * 512:(n + 1) * 512],
                            in0=pss[n], in1=bb_bc,
                            op=mybir.AluOpType.add,
                        )

                qk_proj(wk_sb, xk_sb, bk_sb, khT, 0)
                # v-projection (bf16)
                for kt in range(KT):
                    ps = pj.tile([128, 256], F32, tag="pv", name="psv")
                    for c in range(CT):
                        nc.tensor.matmul(
                            ps,
                            xv_sb[:, c, kt * 128:(kt + 1) * 128],
                            wv_sb[:, c, :],
                            start=(c == 0),
                            stop=(c == CT - 1),
                        )
                    for h in range(HPC):
                        p, is_b = h // 2, h % 2
                        col = p * 193 + (129 if is_b else 0)
                        nc.vector.tensor_tensor(
                            out=vha[kt][:, col:col + 64],
                            in0=ps[:, h * 64:(h + 1) * 64],
                            in1=bvb_sb[:, h * 64:(h + 1) * 64],
                            op=mybir.AluOpType.add,
                        )
                qk_proj(wk_sb, xk_sb, bk_sb, khT, 1)
                qk_proj(wq_sb, xq_sb, bq_sb, qhT, 0)
                qk_proj(wq_sb, xq_sb, bq_sb, qhT, 1)
                # (m is the head-pair index; both m needed before attention)

            # ---- attention + pipelined normalization + oproj ----
            with tc.tile_pool(name="scp", bufs=2, space="PSUM") as scp, \
                 tc.tile_pool(name="pvp", bufs=2, space="PSUM") as pvp:

                def emit_norm_head(qt, po):
                    """Steps at qt end: stage sums + free po via plain copies."""
                    qsl = slice(qt * 512, (qt + 1) * 512)
                    for p in range(2):
                        nc.vector.tensor_copy(
                            out=sums_st[64:65, p, :], in_=po[p][64:65, 0:512])
                        nc.vector.tensor_copy(
                            out=sums_st[32:33, p, :], in_=po[p][32:33, 512:1024])
                    for p in range(2):
                        nc.vector.tensor_copy(
                            out=OT[p][0:64, qsl], in_=po[p][0:64, 0:512])
                        nc.vector.tensor_copy(
                            out=OT[p][64:128, qsl], in_=po[p][64:128, 512:1024])

                def make_norm_tail(qt):
                    """Deferred steps, interleaved into the next qt."""
                    qsl = slice(qt * 512, (qt + 1) * 512)
                    rec_in = singles.tile([2, 2, 512], F32, tag="rec_in",
                                          name="rec_in", bufs=2)
                    rec_f = singles.tile([2, 2, 512], F32, tag="rec_f",
                                         name="rec_f", bufs=2)
                    rec_bf = singles.tile([2, 2, 512], BF16, tag="rec_bf",
                                          name="rec_bf", bufs=2)
                    rbc = [None, None]

                    def s_gather():
                        s32 = sums_st[32:33, :, :]
                        s64 = sums_st[64:65, :, :]
                        src = bass.AP(
                            tensor=s32.tensor, offset=s32.offset,
                            ap=[[s64.offset - s32.offset, 2],
                                list(s32.ap[1]), list(s32.ap[2])])
                        nc.sync.dma_start(out=rec_in, in_=src)

                    def s_recip():
                        nc.vector.reciprocal_approx_fast(out=rec_f, in_=rec_in)
                        nc.vector.tensor_copy(out=rec_bf, in_=rec_f)

                    def s_bcast(p):
                        def go():
                            rps = scp.tile([128, 1024], F32, tag="sc", name="rps")
                            nc.tensor.matmul(
                                rps[:, 0:512], sel_sb, rec_bf[:, p, :],
                                start=True, stop=True)
                            rb = singles.tile([128, 512], BF16, tag=f"rbc{p}",
                                              name=f"rbc{p}", bufs=2)
                            nc.vector.tensor_copy(out=rb, in_=rps[:, 0:512])
                            rbc[p] = rb
                        return go

                    def s_scale(p, is_b):
                        def go():
                            rows = slice(64, 128) if is_b else slice(0, 64)
                            nc.vector.tensor_tensor(
                                out=OT[p][rows, qsl], in0=OT[p][rows, qsl],
                                in1=rbc[p][rows, :],
                                op=mybir.AluOpType.mult,
                            )
                        return go

                    return [s_gather, s_recip, s_bcast(0), s_bcast(1),
                            s_scale(0, 0), s_scale(0, 1),
                            s_scale(1, 0), s_scale(1, 1)]

                pending = []
                for qt in range(QT):
                    po = [pvp.tile([128, 1024], F32, tag="po", name="po",
                                   bufs=2) for _ in range(2)]
                    for kt in range(KT):
                        mt = singles.tile([128, 512], BF16, tag="mask",
                                          name="mask", bufs=6)
                        nc.sync.dma_start(out=mt, in_=mk_d[kt, qt])
                        m_ap = mt[:, :]
                        mbc = bass.AP(
                            tensor=m_ap.tensor,
                            offset=m_ap.offset,
                            ap=[list(m_ap.ap[0]), [0, 2], list(m_ap.ap[1])],
                        )
                        for p in range(2):
                            ps = scp.tile([128, 1024], F32, tag="sc", name="ps")
                            for ab in range(2):
                                nc.tensor.matmul(
                                    ps[:, ab * 512:(ab + 1) * 512],
                                    khT[p][ab * 64:(ab + 1) * 64,
                                           kt * 128:(kt + 1) * 128],
                                    qhT[p][ab * 64:(ab + 1) * 64,
                                           qt * 512:(qt + 1) * 512],
                                    start=True,
                                    stop=True,
                                )
                            pt = singles.tile([128, 1024], BF16, tag="pt",
                                              name="pt", bufs=8)
                            nc.scalar.activation(
                                out=pt, in_=ps,
                                func=mybir.ActivationFunctionType.Exp,
                                scale=float(SCALE),
                            )
                            nc.vector.tensor_tensor(
                                out=pt, in0=pt, in1=mbc,
                                op=mybir.AluOpType.mult,
                            )
                            base = p * 193
                            nc.tensor.matmul(
                                po[p][0:65, 0:512],
                                vha[kt][:, base:base + 65],
                                pt[:, 0:512],
                                start=(kt == 0), stop=(kt == KT - 1),
                            )
                            nc.tensor.matmul(
                                po[p][:, 512:1024],
                                vha[kt][:, base + 65:base + 193],
                                pt[:, 512:1024],
                                start=(kt == 0), stop=(kt == KT - 1),
                            )
                        if pending:
                            pending.pop(0)()
                    while pending:
                        pending.pop(0)()
                    emit_norm_head(qt, po)
                    pending = make_norm_tail(qt)

                # ---- output projection (reuses scp rotation; PE stays hot).
                # qt3's norm tail interleaves into the half0 emissions so the
                # oproj psum slots are requested before the rps broadcasts.
                for half in range(2):
                    for ot in range(8):
                        ps = scp.tile([128, 1024], F32, tag="sc", name="psy")
                        for p in range(2):
                            for n in range(2):
                                nc.tensor.matmul(
                                    ps[:, n * 512:(n + 1) * 512],
                                    wo_sb[p][:, ot * 128:(ot + 1) * 128],
                                    OT[p][:, (half * 2 + n) * 512:
                                          (half * 2 + n + 1) * 512],
                                    start=(p == 0),
                                    stop=(p == 1),
                                )
                        yt = singles.tile([128, 1024], BF16, tag="yt",
                                          name="yt", bufs=4)
                        if (half * 8 + ot) % 2 == 0:
                            nc.scalar.copy(out=yt, in_=ps)
                        else:
                            nc.vector.tensor_copy(out=yt, in_=ps)
                        nc.sync.dma_start(
                            out=yt_d[ot][:, half * 1024:(half + 1) * 1024],
                            in_=yt)
                        if pending:
                            pending.pop(0)()
    nc.compile()
    return nc


_NC_CACHE = None


def get_nc():
    global _NC_CACHE
    if _NC_CACHE is None:
        _NC_CACHE = build_nc()
    return _NC_CACHE


def _tile_ct(xT):
    # [1024, N] -> [128, CT, N]  (c-block-major partition layout)
    n = xT.shape[1]
    return np.ascontiguousarray(xT.reshape(CT, 128, n).transpose(1, 0, 2))


def prep_in_maps(q, k, v, mask, Wq, bq, Wk, bk, Wv, bv, Wo, bo):
    q = np.asarray(q, np.float32)
    k = np.asarray(k, np.float32)
    v = np.asarray(v, np.float32)
    mask = np.asarray(mask)
    WqT = np.asarray(Wq, np.float32).T
    WkT = np.asarray(Wk, np.float32).T
    WvT = np.asarray(Wv, np.float32).T
    WoT = np.asarray(Wo, np.float32).T
    bq = np.asarray(bq, np.float32)
    bk = np.asarray(bk, np.float32)
    bv = np.asarray(bv, np.float32)

    xT = {}
    keepT = {}
    for b in range(B):
        xT[b] = (
            _tile_ct(np.ascontiguousarray(q[b].T)).astype(NP_F8),
            _tile_ct(np.ascontiguousarray(k[b].T)).astype(NP_F8),
            _tile_ct(np.ascontiguousarray(v[b].T)).astype(NP_BF16),
        )
        mt = np.ascontiguousarray((~mask[b, 0]).T.astype(np.float32)).astype(NP_BF16)
        keepT[b] = np.ascontiguousarray(
            mt.reshape(KT, 128, QT, 512).transpose(0, 2, 1, 3))

    sel = np.zeros((2, 128), np.float32)
    sel[0, 64:128] = 1.0  # row0 <- partition-32 (B) sums -> dims 64:128
    sel[1, 0:64] = 1.0    # row1 <- partition-64 (A) sums -> dims 0:64
    sel = sel.astype(NP_BF16)

    in_maps = []
    for c in range(N_CORES):
        b = c // 4
        ho = c % 4
        dsl = slice(ho * 256, ho * 256 + 256)
        xq, xk, xv = xT[b]
        in_maps.append({
            "xq": xq,
            "xk": xk,
            "xv": xv,
            "wq": _tile_ct(np.ascontiguousarray(WqT[:, dsl])).astype(NP_F8),
            "wk": _tile_ct(np.ascontiguousarray(WkT[:, dsl])).astype(NP_F8),
            "wv": _tile_ct(np.ascontiguousarray(WvT[:, dsl])).astype(NP_BF16),
            "wo": np.ascontiguousarray(WoT[dsl, :]).astype(NP_BF16).reshape(2, 128, 1024),
            "bq2": np.ascontiguousarray(bq[dsl]).reshape(2, 128, 1).astype(np.float32),
            "bk2": np.ascontiguousarray(bk[dsl]).reshape(2, 128, 1).astype(np.float32),
            "bvb": np.ascontiguousarray(
                np.broadcast_to(bv[dsl], (128, 256))).astype(NP_BF16),
            "mk": keepT[b],
            "selc": sel,
        })
    return in_maps


def gather_output(results, bo):
    bo = np.asarray(bo, np.float32)
    y = np.zeros((B, S, DIM), np.float32)
    for c in range(N_CORES):
        yt = np.asarray(results[c]["yt"], np.float32)  # [8, 128, 2048]
        yT = yt.reshape(DIM, S)
        y[c // 4] += yT.T
    y += bo[None, None, :]
    return y


def kernel(**inputs):
    nc = get_nc()
    in_maps = prep_in_maps(**{k_: inputs[k_] for k_ in (
        "q", "k", "v", "mask", "Wq", "bq", "Wk", "bk", "Wv", "bv", "Wo", "bo")})
    res = bass_utils.run_bass_kernel_spmd(nc, in_maps, list(range(N_CORES)))
    return gather_output(res.results, inputs["bo"])


# revision 25
# speedup vs baseline: 1.2011x; 1.0135x over previous
"""MultiHeadAttention Trainium2 Bass kernel (8-core SPMD), v2.

Problem: B=2, S=2048, DIM=1024, H=16 heads (dh=64), fp32 reference.
Sharding: core c handles batch b = c//4 and 4 heads ho = 4*(c%4)..+4.

v2 changes vs v1 (332us -> target ~180us):
- q/k path in fp8e4m3 (x AND W, host-validated rel-err 3.6e-3 vs 2e-2
  budget); v path stays bf16 (fp8 there costs 1.3e-2).
- q/k projections use MatmulPerfMode.DoubleRow (fp8 K=256 contraction).
- x / W tensors shipped as single [128, CT, *] tiles -> one large DMA
  each with 4-32KB per-partition contiguous runs (~350+ GB/s vs ~200).
- Per-qt pipelined softmax normalization: sums -> recip -> selector-
  matmul partition-broadcast -> in-place OT scale, all hidden under the
  next qt's attention (replaces a 20us end-of-kernel stall + DRAM
  round-trips).
- Output projection PSUM reuses the scores pool rotation so the PE
  never idles between attention and oproj (HAM stays warm); PSUM->SBUF
  output copies alternate ACT/DVE.
- Attention phase is ACT(exp)-bound (~1us per [128,1024] exp); PE/DVE
  loads are kept strictly below that (~0.86us / ~0.85us per half-tile).
"""

import os
import sys

sys.path.insert(0, "/opt/trn_rl_repo")
os.environ.setdefault("MYCRO_LOCAL_CACHE", "1")

import numpy as np

import concourse.bass as bass
import concourse.bacc as bacc
import concourse.tile as tile
from concourse import mybir
from concourse import bass_utils

F32 = mybir.dt.float32
BF16 = mybir.dt.bfloat16
F8 = mybir.dt.float8e4
NP_BF16 = mybir.dt.np(BF16)
NP_F8 = mybir.dt.np(F8)
DR = mybir.MatmulPerfMode.DoubleRow

B, S, DIM = 2, 2048, 1024
H = 16
DH = 64
SCALE = 1.0 / (DIM ** 0.5)
N_CORES = 8
HPC = 4          # heads per core
QT = S // 512    # 4 q-chunks of 512
KT = S // 128    # 16 k-tiles of 128
CT = DIM // 128  # 8 contraction tiles for projections

# vh_aug per-kt layout (unchanged from v1): per pair p (2 local pairs):
#   A block: [vh_A(64) | ones(1)]                 at cols p*193 + [0, 65)
#   B block: [zeros(32) | ones(1) | zeros(31) | vh_B(64)] at cols p*193 + [65, 193)
VHA_W = 386


def build_nc():
    nc = bacc.Bacc("TRN2", target_bir_lowering=False)

    xq_d = nc.declare_dram_parameter("xq", [128, CT, S], F8, isOutput=False)
    xk_d = nc.declare_dram_parameter("xk", [128, CT, S], F8, isOutput=False)
    xv_d = nc.declare_dram_parameter("xv", [128, CT, S], BF16, isOutput=False)
    wq_d = nc.declare_dram_parameter("wq", [128, CT, 256], F8, isOutput=False)
    wk_d = nc.declare_dram_parameter("wk", [128, CT, 256], F8, isOutput=False)
    wv_d = nc.declare_dram_parameter("wv", [128, CT, 256], BF16, isOutput=False)
    wo_d = nc.declare_dram_parameter("wo", [2, 128, 1024], BF16, isOutput=False)
    bq_d = nc.declare_dram_parameter("bq2", [2, 128, 1], F32, isOutput=False)
    bk_d = nc.declare_dram_parameter("bk2", [2, 128, 1], F32, isOutput=False)
    bvb_d = nc.declare_dram_parameter("bvb", [128, 256], BF16, isOutput=False)
    mk_d = nc.declare_dram_parameter("mk", [KT, QT, 128, 512], BF16, isOutput=False)
    sel_d = nc.declare_dram_parameter("selc", [2, 128], BF16, isOutput=False)
    yt_d = nc.declare_dram_parameter("yt", [8, 128, 2048], BF16, isOutput=True)

    with tile.TileContext(nc) as tc:
        with tc.tile_pool(name="persist", bufs=1) as singles:
            # ---- small operands first on the DMA queue ----
            bq_sb, bk_sb = [], []
            for m in range(2):
                tq = singles.tile([128, 1], F32, tag=f"bq{m}", name=f"bq{m}")
                nc.sync.dma_start(out=tq, in_=bq_d[m])
                bq_sb.append(tq)
                tk = singles.tile([128, 1], F32, tag=f"bk{m}", name=f"bk{m}")
                nc.sync.dma_start(out=tk, in_=bk_d[m])
                bk_sb.append(tk)
            bvb_sb = singles.tile([128, 256], BF16, tag="bvb")
            nc.sync.dma_start(out=bvb_sb, in_=bvb_d[:, :])

            wv_sb = singles.tile([128, CT, 256], BF16, tag="wv", name="wv")
            nc.sync.dma_start(out=wv_sb, in_=wv_d[:, :, :])
            wk_sb = singles.tile([128, CT, 256], F8, tag="wk", name="wk")
            nc.sync.dma_start(out=wk_sb, in_=wk_d[:, :, :])
            wq_sb = singles.tile([128, CT, 256], F8, tag="wq", name="wq")
            nc.sync.dma_start(out=wq_sb, in_=wq_d[:, :, :])
            wo_sb = []
            for m in range(2):
                t = singles.tile([128, 1024], BF16, tag=f"wo{m}", name=f"wo{m}")
                nc.sync.dma_start(out=t, in_=wo_d[m])
                wo_sb.append(t)

            # ---- bulk x: v first (v-proj is the long LDW-bound pole) ----
            xv_sb = singles.tile([128, CT, S], BF16, tag="xv", name="xv")
            nc.sync.dma_start(out=xv_sb, in_=xv_d[:, :, :])
            xk_sb = singles.tile([128, CT, S], F8, tag="xk", name="xk")
            nc.sync.dma_start(out=xk_sb, in_=xk_d[:, :, :])
            xq_sb = singles.tile([128, CT, S], F8, tag="xq", name="xq")
            nc.sync.dma_start(out=xq_sb, in_=xq_d[:, :, :])

            # ---- persistent intermediates ----
            qhT = [singles.tile([128, S], BF16, tag=f"qhT{m}", name=f"qhT{m}")
                   for m in range(2)]
            khT = [singles.tile([128, S], BF16, tag=f"khT{m}", name=f"khT{m}")
                   for m in range(2)]
            OT = [singles.tile([128, S], BF16, tag=f"OT{m}", name=f"OT{m}")
                  for m in range(2)]
            vha = [singles.tile([128, VHA_W], BF16, tag=f"vha{kt}",
                                name=f"vha{kt}") for kt in range(KT)]
            sums_st = singles.tile([128, 2, 512], F32, tag="sums_st")
            sel_sb = singles.tile([2, 128], BF16, tag="sel")

            # warm tile memset FIRST: the PE warmup gates on it, and the
            # vha/sums memsets behind it cost ~15us of gpsimd time.
            warm = singles.tile([128, 512], BF16, tag="warm")
            nc.gpsimd.memset(warm[:, :], 0.0)
            nc.gpsimd.memset(sums_st[:, :, :], 1.0)
            for kt in range(KT):
                for p in range(2):
                    base = p * 193
                    nc.gpsimd.memset(vha[kt][:, base + 64:base + 65], 1.0)
                    nc.gpsimd.memset(vha[kt][:, base + 97:base + 98], 1.0)
                    nc.gpsimd.memset(vha[kt][:, base + 65:base + 97], 0.0)
                    nc.gpsimd.memset(vha[kt][:, base + 98:base + 129], 0.0)
            # selector: row0 (gathered from partition 32) = B sums -> dims 64:128
            #           row1 (partition 64) = A sums -> dims 0:64
            nc.sync.dma_start(out=sel_sb, in_=sel_d[:, :])

            # ---- projections ----
            with tc.tile_pool(name="pjp", bufs=2, space="PSUM") as pj:
                # PE warmup to open the HAM clock gate while DMAs land
                wps = pj.tile([128, 512], F32, tag="pwarm", name="wps")
                for i in range(34):
                    nc.tensor.matmul(
                        wps, warm[:, 0:128], warm[:, :],
                        start=True, stop=True)

                def qk_proj(w_sb, x_sb, b_sb, dst, m):
                    # weight (c-pair, m-half) stays stationary across the 4
                    # n-chunks -> DoubleRow LDWEIGHTS amortized 4x.
                    pss = [pj.tile([128, 512], F32, tag=f"pqk{n}",
                                   name=f"psqk{n}", bufs=1) for n in range(4)]
                    for ci in range(4):
                        for n in range(4):
                            nc.tensor.matmul(
                                pss[n],
                                w_sb[:, 2 * ci:2 * ci + 2, m * 128:(m + 1) * 128],
                                x_sb[:, 2 * ci:2 * ci + 2, n * 512:(n + 1) * 512],
                                start=(ci == 0),
                                stop=(ci == 3),
                                perf_mode=DR,
                            )
                    bb = b_sb[m][:, 0:1]
                    bb_bc = bass.AP(
                        tensor=bb.tensor, offset=bb.offset,
                        ap=[list(bb.ap[0]), [0, 512]])
                    for n in range(4):
                        nc.vector.tensor_tensor(
                            out=dst[m][:, n * 512:(n + 1) * 512],
                            in0=pss[n], in1=bb_bc,
                            op=mybir.AluOpType.add,
                        )

                # v-projection (bf16) first: xv arrives first
                for kt in range(KT):
                    ps = pj.tile([128, 256], F32, tag="pv", name="psv")
                    for c in range(CT):
                        nc.tensor.matmul(
                            ps,
                            xv_sb[:, c, kt * 128:(kt + 1) * 128],
                            wv_sb[:, c, :],
                            start=(c == 0),
                            stop=(c == CT - 1),
                        )
                    for h in range(HPC):
                        p, is_b = h // 2, h % 2
                        col = p * 193 + (129 if is_b else 0)
                        nc.vector.tensor_tensor(
                            out=vha[kt][:, col:col + 64],
                            in0=ps[:, h * 64:(h + 1) * 64],
                            in1=bvb_sb[:, h * 64:(h + 1) * 64],
                            op=mybir.AluOpType.add,
                        )
                qk_proj(wk_sb, xk_sb, bk_sb, khT, 0)
                qk_proj(wk_sb, xk_sb, bk_sb, khT, 1)
                qk_proj(wq_sb, xq_sb, bq_sb, qhT, 0)
                qk_proj(wq_sb, xq_sb, bq_sb, qhT, 1)
                # (m is the head-pair index; both m needed before attention)

            # ---- attention + pipelined normalization + oproj ----
            with tc.tile_pool(name="scp", bufs=2, space="PSUM") as scp, \
                 tc.tile_pool(name="pvp", bufs=2, space="PSUM") as pvp:

                def emit_norm_head(qt, po):
                    """Steps at qt end: free po (OT copies) + stage sums."""
                    qsl = slice(qt * 512, (qt + 1) * 512)
                    for p in range(2):
                        nc.vector.tensor_copy(
                            out=OT[p][0:64, qsl], in_=po[p][0:64, 0:512])
                        nc.vector.tensor_copy(
                            out=OT[p][64:128, qsl], in_=po[p][64:128, 512:1024])
                    for p in range(2):
                        nc.vector.tensor_copy(
                            out=sums_st[64:65, p, :], in_=po[p][64:65, 0:512])
                        nc.vector.tensor_copy(
                            out=sums_st[32:33, p, :], in_=po[p][32:33, 512:1024])

                def make_norm_tail(qt):
                    """Deferred steps, interleaved into the next qt."""
                    qsl = slice(qt * 512, (qt + 1) * 512)
                    rec_in = singles.tile([2, 2, 512], F32, tag="rec_in",
                                          name="rec_in", bufs=2)
                    rec_f = singles.tile([2, 2, 512], F32, tag="rec_f",
                                         name="rec_f", bufs=2)
                    rec_bf = singles.tile([2, 2, 512], BF16, tag="rec_bf",
                                          name="rec_bf", bufs=2)
                    rbc = [None, None]

                    def s_gather():
                        s32 = sums_st[32:33, :, :]
                        s64 = sums_st[64:65, :, :]
                        src = bass.AP(
                            tensor=s32.tensor, offset=s32.offset,
                            ap=[[s64.offset - s32.offset, 2],
                                list(s32.ap[1]), list(s32.ap[2])])
                        nc.sync.dma_start(out=rec_in, in_=src)

                    def s_recip():
                        nc.vector.reciprocal_approx_fast(out=rec_f, in_=rec_in)
                        nc.vector.tensor_copy(out=rec_bf, in_=rec_f)

                    def s_bcast(p):
                        def go():
                            rps = scp.tile([128, 1024], F32, tag="sc", name="rps")
                            nc.tensor.matmul(
                                rps[:, 0:512], sel_sb, rec_bf[:, p, :],
                                start=True, stop=True)
                            rb = singles.tile([128, 512], BF16, tag=f"rbc{p}",
                                              name=f"rbc{p}", bufs=2)
                            nc.vector.tensor_copy(out=rb, in_=rps[:, 0:512])
                            rbc[p] = rb
                        return go

                    def s_scale(p, is_b):
                        def go():
                            rows = slice(64, 128) if is_b else slice(0, 64)
                            nc.vector.tensor_tensor(
                                out=OT[p][rows, qsl], in0=OT[p][rows, qsl],
                                in1=rbc[p][rows, :],
                                op=mybir.AluOpType.mult,
                            )
                        return go

                    return [s_gather, s_recip, s_bcast(0), s_bcast(1),
                            s_scale(0, 0), s_scale(0, 1),
                            s_scale(1, 0), s_scale(1, 1)]

                def emit_pv(po, kt, pts):
                    for p in range(2):
                        base = p * 193
                        nc.tensor.matmul(
                            po[p][0:65, 0:512],
                            vha[kt][:, base:base + 65],
                            pts[p][:, 0:512],
                            start=(kt == 0), stop=(kt == KT - 1),
                        )
                        nc.tensor.matmul(
                            po[p][:, 512:1024],
                            vha[kt][:, base + 65:base + 193],
                            pts[p][:, 512:1024],
                            start=(kt == 0), stop=(kt == KT - 1),
                        )

                pending = []
                for qt in range(QT):
                    po = [pvp.tile([128, 1024], F32, tag="po", name="po",
                                   bufs=2) for _ in range(2)]
                    # PV matmuls trail the scores by one kt so the PE queue
                    # always holds ready score work when a boundary stalls
                    # the PV chain.
                    prev_pv = None
                    for kt in range(KT):
                        mt = singles.tile([128, 512], BF16, tag="mask",
                                          name="mask", bufs=6)
                        nc.sync.dma_start(out=mt, in_=mk_d[kt, qt])
                        m_ap = mt[:, :]
                        mbc = bass.AP(
                            tensor=m_ap.tensor,
                            offset=m_ap.offset,
                            ap=[list(m_ap.ap[0]), [0, 2], list(m_ap.ap[1])],
                        )
                        pts = []
                        for p in range(2):
                            ps = scp.tile([128, 1024], F32, tag="sc", name="ps")
                            for ab in range(2):
                                nc.tensor.matmul(
                                    ps[:, ab * 512:(ab + 1) * 512],
                                    khT[p][ab * 64:(ab + 1) * 64,
                                           kt * 128:(kt + 1) * 128],
                                    qhT[p][ab * 64:(ab + 1) * 64,
                                           qt * 512:(qt + 1) * 512],
                                    start=True,
                                    stop=True,
                                )
                            pt = singles.tile([128, 1024], BF16, tag="pt",
                                              name="pt", bufs=8)
                            nc.scalar.activation(
                                out=pt, in_=ps,
                                func=mybir.ActivationFunctionType.Exp,
                                scale=float(SCALE),
                            )
                            nc.vector.tensor_tensor(
                                out=pt, in0=pt, in1=mbc,
                                op=mybir.AluOpType.mult,
                            )
                            pts.append(pt)
                        if prev_pv is not None:
                            emit_pv(po, *prev_pv)
                        prev_pv = (kt, pts)
                        if pending:
                            pending.pop(0)()
                    emit_pv(po, *prev_pv)
                    while pending:
                        pending.pop(0)()
                    emit_norm_head(qt, po)
                    pending = make_norm_tail(qt)

                # ---- output projection (reuses scp rotation; PE stays hot).
                # qt3's norm tail interleaves into the half0 emissions so the
                # oproj psum slots are requested before the rps broadcasts.
                for half in range(2):
                    for ot in range(8):
                        idx = half * 8 + ot
                        if idx % 2 == 0:
                            ps = scp.tile([128, 1024], F32, tag="sc", name="psy")
                        else:
                            ps = pvp.tile([128, 1024], F32, tag="po", name="psy")
                        for p in range(2):
                            for n in range(2):
                                nc.tensor.matmul(
                                    ps[:, n * 512:(n + 1) * 512],
                                    wo_sb[p][:, ot * 128:(ot + 1) * 128],
                                    OT[p][:, (half * 2 + n) * 512:
                                          (half * 2 + n + 1) * 512],
                                    start=(p == 0),
                                    stop=(p == 1),
                                )
                        yt = singles.tile([128, 1024], BF16, tag="yt",
                                          name="yt", bufs=4)
                        if idx % 2 == 0:
                            nc.scalar.copy(out=yt, in_=ps)
                            nc.sync.dma_start(
                                out=yt_d[ot][:, half * 1024:(half + 1) * 1024],
                                in_=yt)
                        else:
                            nc.vector.tensor_copy(out=yt, in_=ps)
                            nc.scalar.dma_start(
                                out=yt_d[ot][:, half * 1024:(half + 1) * 1024],
                                in_=yt)
                        if pending:
                            pending.pop(0)()
    nc.compile()
    return nc


_NC_CACHE = None


def get_nc():
    global _NC_CACHE
    if _NC_CACHE is None:
        _NC_CACHE = build_nc()
    return _NC_CACHE


def _tile_ct(xT):
    # [1024, N] -> [128, CT, N]  (c-block-major partition layout)
    n = xT.shape[1]
    return np.ascontiguousarray(xT.reshape(CT, 128, n).transpose(1, 0, 2))


def prep_in_maps(q, k, v, mask, Wq, bq, Wk, bk, Wv, bv, Wo, bo):
    q = np.asarray(q, np.float32)
    k = np.asarray(k, np.float32)
    v = np.asarray(v, np.float32)
    mask = np.asarray(mask)
    WqT = np.asarray(Wq, np.float32).T
    WkT = np.asarray(Wk, np.float32).T
    WvT = np.asarray(Wv, np.float32).T
    WoT = np.asarray(Wo, np.float32).T
    bq = np.asarray(bq, np.float32)
    bk = np.asarray(bk, np.float32)
    bv = np.asarray(bv, np.float32)

    xT = {}
    keepT = {}
    for b in range(B):
        xT[b] = (
            _tile_ct(np.ascontiguousarray(q[b].T)).astype(NP_F8),
            _tile_ct(np.ascontiguousarray(k[b].T)).astype(NP_F8),
            _tile_ct(np.ascontiguousarray(v[b].T)).astype(NP_BF16),
        )
        mt = np.ascontiguousarray((~mask[b, 0]).T.astype(np.float32)).astype(NP_BF16)
        keepT[b] = np.ascontiguousarray(
            mt.reshape(KT, 128, QT, 512).transpose(0, 2, 1, 3))

    sel = np.zeros((2, 128), np.float32)
    sel[0, 64:128] = 1.0  # row0 <- partition-32 (B) sums -> dims 64:128
    sel[1, 0:64] = 1.0    # row1 <- partition-64 (A) sums -> dims 0:64
    sel = sel.astype(NP_BF16)

    in_maps = []
    for c in range(N_CORES):
        b = c // 4
        ho = c % 4
        dsl = slice(ho * 256, ho * 256 + 256)
        xq, xk, xv = xT[b]
        in_maps.append({
            "xq": xq,
            "xk": xk,
            "xv": xv,
            "wq": _tile_ct(np.ascontiguousarray(WqT[:, dsl])).astype(NP_F8),
            "wk": _tile_ct(np.ascontiguousarray(WkT[:, dsl])).astype(NP_F8),
            "wv": _tile_ct(np.ascontiguousarray(WvT[:, dsl])).astype(NP_BF16),
            "wo": np.ascontiguousarray(WoT[dsl, :]).astype(NP_BF16).reshape(2, 128, 1024),
            "bq2": np.ascontiguousarray(bq[dsl]).reshape(2, 128, 1).astype(np.float32),
            "bk2": np.ascontiguousarray(bk[dsl]).reshape(2, 128, 1).astype(np.float32),
            "bvb": np.ascontiguousarray(
                np.broadcast_to(bv[dsl], (128, 256))).astype(NP_BF16),
            "mk": keepT[b],
            "selc": sel,
        })
    return in_maps


def gather_output(results, bo):
    bo = np.asarray(bo, np.float32)
    y = np.zeros((B, S, DIM), np.float32)
    for c in range(N_CORES):
        yt = np.asarray(results[c]["yt"], np.float32)  # [8, 128, 2048]
        yT = yt.reshape(DIM, S)
        y[c // 4] += yT.T
    y += bo[None, None, :]
    return y


def kernel(**inputs):
    nc = get_nc()
    in_maps = prep_in_maps(**{k_: inputs[k_] for k_ in (
        "q", "k", "v", "mask", "Wq", "bq", "Wk", "bk", "Wv", "bv", "Wo", "bo")})
    res = bass_utils.run_bass_kernel_spmd(nc, in_maps, list(range(N_CORES)))
    return gather_output(res.results, inputs["bo"])


# revision 33
# speedup vs baseline: 1.2300x; 1.0240x over previous
"""MultiHeadAttention Trainium2 Bass kernel (8-core SPMD), v2.

Problem: B=2, S=2048, DIM=1024, H=16 heads (dh=64), fp32 reference.
Sharding: core c handles batch b = c//4 and 4 heads ho = 4*(c%4)..+4.

v2 changes vs v1 (332us -> target ~180us):
- q/k path in fp8e4m3 (x AND W, host-validated rel-err 3.6e-3 vs 2e-2
  budget); v path stays bf16 (fp8 there costs 1.3e-2).
- q/k projections use MatmulPerfMode.DoubleRow (fp8 K=256 contraction).
- x / W tensors shipped as single [128, CT, *] tiles -> one large DMA
  each with 4-32KB per-partition contiguous runs (~350+ GB/s vs ~200).
- Per-qt pipelined softmax normalization: sums -> recip -> selector-
  matmul partition-broadcast -> in-place OT scale, all hidden under the
  next qt's attention (replaces a 20us end-of-kernel stall + DRAM
  round-trips).
- Output projection PSUM reuses the scores pool rotation so the PE
  never idles between attention and oproj (HAM stays warm); PSUM->SBUF
  output copies alternate ACT/DVE.
- Attention phase is ACT(exp)-bound (~1us per [128,1024] exp); PE/DVE
  loads are kept strictly below that (~0.86us / ~0.85us per half-tile).
"""

import os
import sys

sys.path.insert(0, "/opt/trn_rl_repo")
os.environ.setdefault("MYCRO_LOCAL_CACHE", "1")

import numpy as np

import concourse.bass as bass
import concourse.bacc as bacc
import concourse.tile as tile
from concourse import mybir
from concourse import bass_utils

F32 = mybir.dt.float32
BF16 = mybir.dt.bfloat16
F8 = mybir.dt.float8e4
NP_BF16 = mybir.dt.np(BF16)
NP_F8 = mybir.dt.np(F8)
DR = mybir.MatmulPerfMode.DoubleRow

B, S, DIM = 2, 2048, 1024
H = 16
DH = 64
SCALE = 1.0 / (DIM ** 0.5)
N_CORES = 8
HPC = 4          # heads per core
QT = S // 512    # 4 q-chunks of 512
KT = S // 128    # 16 k-tiles of 128
CT = DIM // 128  # 8 contraction tiles for projections

# vh_aug per-kt layout (unchanged from v1): per pair p (2 local pairs):
#   A block: [vh_A(64) | ones(1)]                 at cols p*193 + [0, 65)
#   B block: [zeros(32) | ones(1) | zeros(31) | vh_B(64)] at cols p*193 + [65, 193)
VHA_W = 386


def build_nc():
    nc = bacc.Bacc("TRN2", target_bir_lowering=False)

    xq_d = nc.declare_dram_parameter("xq", [128, CT, S], F8, isOutput=False)
    xk_d = nc.declare_dram_parameter("xk", [128, CT, S], F8, isOutput=False)
    xv_d = nc.declare_dram_parameter("xv", [128, CT, S], BF16, isOutput=False)
    wq_d = nc.declare_dram_parameter("wq", [128, CT, 256], F8, isOutput=False)
    wk_d = nc.declare_dram_parameter("wk", [128, CT, 256], F8, isOutput=False)
    wv_d = nc.declare_dram_parameter("wv", [128, CT, 256], BF16, isOutput=False)
    wo_d = nc.declare_dram_parameter("wo", [2, 128, 1024], BF16, isOutput=False)
    # packed biases: cols 0:2 = bq halves, 2:4 = bk halves, 4:260 = bv bcast
    bc_d = nc.declare_dram_parameter("bcom", [128, 260], F32, isOutput=False)
    mk_d = nc.declare_dram_parameter("mk", [KT, QT, 128, 512], BF16, isOutput=False)
    sel_d = nc.declare_dram_parameter("selc", [2, 128], BF16, isOutput=False)
    yt_d = nc.declare_dram_parameter("yt", [8, 128, 2048], BF16, isOutput=True)

    with tile.TileContext(nc) as tc:
        with tc.tile_pool(name="persist", bufs=1) as singles:
            # ---- small operands first on the DMA queue (packed biases) ----
            bc_sb = singles.tile([128, 260], F32, tag="bcom", name="bcom")
            nc.sync.dma_start(out=bc_sb, in_=bc_d[:, :])
            bq_sb = [bc_sb[:, m:m + 1] for m in range(2)]
            bk_sb = [bc_sb[:, 2 + m:3 + m] for m in range(2)]
            bvb_sb = bc_sb[:, 4:260]

            wk_sb = singles.tile([128, CT, 256], F8, tag="wk", name="wk")
            nc.sync.dma_start(out=wk_sb, in_=wk_d[:, :, :])
            wq_sb = singles.tile([128, CT, 256], F8, tag="wq", name="wq")
            nc.sync.dma_start(out=wq_sb, in_=wq_d[:, :, :])
            wv_sb = singles.tile([128, CT, 256], BF16, tag="wv", name="wv")
            nc.sync.dma_start(out=wv_sb, in_=wv_d[:, :, :])
            wo_sb = []
            for m in range(2):
                t = singles.tile([128, 1024], BF16, tag=f"wo{m}", name=f"wo{m}")
                nc.sync.dma_start(out=t, in_=wo_d[m])
                wo_sb.append(t)

            # ---- bulk x in consumption order: k, v, q ----
            xk_sb = singles.tile([128, CT, S], F8, tag="xk", name="xk")
            nc.sync.dma_start(out=xk_sb, in_=xk_d[:, :, :])
            xv_sb = singles.tile([128, CT, S], BF16, tag="xv", name="xv")
            nc.sync.dma_start(out=xv_sb, in_=xv_d[:, :, :])
            xq_sb = singles.tile([128, CT, S], F8, tag="xq", name="xq")
            nc.sync.dma_start(out=xq_sb, in_=xq_d[:, :, :])

            # ---- persistent intermediates ----
            qhT = [singles.tile([128, S], BF16, tag=f"qhT{m}", name=f"qhT{m}")
                   for m in range(2)]
            khT = [singles.tile([128, S], BF16, tag=f"khT{m}", name=f"khT{m}")
                   for m in range(2)]
            OT = [singles.tile([128, S], BF16, tag=f"OT{m}", name=f"OT{m}")
                  for m in range(2)]
            vha = [singles.tile([128, VHA_W], BF16, tag=f"vha{kt}",
                                name=f"vha{kt}") for kt in range(KT)]
            sums_st = singles.tile([128, 2, 512], F32, tag="sums_st")
            sel_sb = singles.tile([2, 128], BF16, tag="sel")

            # warm tile memset FIRST: the PE warmup gates on it, and the
            # vha/sums memsets behind it cost ~15us of gpsimd time.
            warm = singles.tile([128, 512], BF16, tag="warm")
            nc.gpsimd.memset(warm[:, :], 0.0)
            nc.gpsimd.memset(sums_st[:, :, :], 1.0)
            for kt in range(KT):
                for p in range(2):
                    base = p * 193
                    nc.gpsimd.memset(vha[kt][:, base + 64:base + 65], 1.0)
                    nc.gpsimd.memset(vha[kt][:, base + 97:base + 98], 1.0)
                    nc.gpsimd.memset(vha[kt][:, base + 65:base + 97], 0.0)
                    nc.gpsimd.memset(vha[kt][:, base + 98:base + 129], 0.0)
            # selector: row0 (gathered from partition 32) = B sums -> dims 64:128
            #           row1 (partition 64) = A sums -> dims 0:64
            nc.sync.dma_start(out=sel_sb, in_=sel_d[:, :])

            # ---- projections ----
            with tc.tile_pool(name="pjp", bufs=2, space="PSUM") as pj:
                # PE warmup to open the HAM clock gate while DMAs land
                wps = pj.tile([128, 512], F32, tag="pwarm", name="wps")
                for i in range(34):
                    nc.tensor.matmul(
                        wps, warm[:, 0:128], warm[:, :],
                        start=True, stop=True)

                def qk_proj(w_sb, x_sb, b_sb, dst, m):
                    # weight (c-pair, m-half) stays stationary across the 4
                    # n-chunks -> DoubleRow LDWEIGHTS amortized 4x.
                    pss = [pj.tile([128, 512], F32, tag=f"pqk{n}",
                                   name=f"psqk{n}", bufs=1) for n in range(4)]
                    for ci in range(4):
                        for n in range(4):
                            nc.tensor.matmul(
                                pss[n],
                                w_sb[:, 2 * ci:2 * ci + 2, m * 128:(m + 1) * 128],
                                x_sb[:, 2 * ci:2 * ci + 2, n * 512:(n + 1) * 512],
                                start=(ci == 0),
                                stop=(ci == 3),
                                perf_mode=DR,
                            )
                    bb = b_sb[m][:, 0:1]
                    bb_bc = bass.AP(
                        tensor=bb.tensor, offset=bb.offset,
                        ap=[list(bb.ap[0]), [0, 512]])
                    for n in range(4):
                        nc.vector.tensor_tensor(
                            out=dst[m][:, n * 512:(n + 1) * 512],
                            in0=pss[n], in1=bb_bc,
                            op=mybir.AluOpType.add,
                        )

                qk_proj(wk_sb, xk_sb, bk_sb, khT, 0)
                qk_proj(wk_sb, xk_sb, bk_sb, khT, 1)
                # v-projection (bf16)
                for kt in range(KT):
                    ps = pj.tile([128, 256], F32, tag="pv", name="psv")
                    for c in range(CT):
                        nc.tensor.matmul(
                            ps,
                            xv_sb[:, c, kt * 128:(kt + 1) * 128],
                            wv_sb[:, c, :],
                            start=(c == 0),
                            stop=(c == CT - 1),
                        )
                    for h in range(HPC):
                        p, is_b = h // 2, h % 2
                        col = p * 193 + (129 if is_b else 0)
                        nc.vector.tensor_tensor(
                            out=vha[kt][:, col:col + 64],
                            in0=ps[:, h * 64:(h + 1) * 64],
                            in1=bvb_sb[:, h * 64:(h + 1) * 64],
                            op=mybir.AluOpType.add,
                        )
                qk_proj(wq_sb, xq_sb, bq_sb, qhT, 0)
                qk_proj(wq_sb, xq_sb, bq_sb, qhT, 1)
                # (m is the head-pair index; both m needed before attention)

            # ---- attention + pipelined normalization + oproj ----
            with tc.tile_pool(name="scp", bufs=2, space="PSUM") as scp, \
                 tc.tile_pool(name="pvp", bufs=2, space="PSUM") as pvp:

                def make_norm_tail(qt, po):
                    """All qt-end work, split into ~1.4us steps interleaved
                    between the next qt's kt iterations (or oproj tiles).
                    po[p] frees after steps 2p and 2p+1."""
                    qsl = slice(qt * 512, (qt + 1) * 512)
                    rec_in = singles.tile([2, 2, 512], F32, tag="rec_in",
                                          name="rec_in", bufs=2)
                    rec_f = singles.tile([2, 2, 512], F32, tag="rec_f",
                                         name="rec_f", bufs=2)
                    rec_bf = singles.tile([2, 2, 512], BF16, tag="rec_bf",
                                          name="rec_bf", bufs=2)
                    rbc = [None, None]

                    def s_po(p):
                        # OT casts on DVE, sum staging on ACT (parallel);
                        # po[p] frees when all four complete.
                        def go():
                            nc.vector.tensor_copy(
                                out=OT[p][0:64, qsl], in_=po[p][0:64, 0:512])
                            nc.scalar.copy(
                                out=sums_st[64:65, p, :], in_=po[p][64:65, 0:512])
                            nc.vector.tensor_copy(
                                out=OT[p][64:128, qsl],
                                in_=po[p][64:128, 512:1024])
                            nc.scalar.copy(
                                out=sums_st[32:33, p, :],
                                in_=po[p][32:33, 512:1024])
                        return go

                    def s_gather_recip():
                        s32 = sums_st[32:33, :, :]
                        s64 = sums_st[64:65, :, :]
                        src = bass.AP(
                            tensor=s32.tensor, offset=s32.offset,
                            ap=[[s64.offset - s32.offset, 2],
                                list(s32.ap[1]), list(s32.ap[2])])
                        nc.sync.dma_start(out=rec_in, in_=src)
                        nc.vector.reciprocal_approx_fast(out=rec_f, in_=rec_in)
                        nc.vector.tensor_copy(out=rec_bf, in_=rec_f)

                    def s_bcast(p):
                        def go():
                            rps = scp.tile([128, 1024], F32, tag="sc", name="rps")
                            nc.tensor.matmul(
                                rps[:, 0:512], sel_sb, rec_bf[:, p, :],
                                start=True, stop=True)
                            rb = singles.tile([128, 512], BF16, tag=f"rbc{p}",
                                              name=f"rbc{p}", bufs=2)
                            nc.vector.tensor_copy(out=rb, in_=rps[:, 0:512])
                            rbc[p] = rb
                        return go

                    def s_scale(p):
                        def go():
                            nc.vector.tensor_tensor(
                                out=OT[p][0:64, qsl], in0=OT[p][0:64, qsl],
                                in1=rbc[p][0:64, :],
                                op=mybir.AluOpType.mult,
                            )
                            nc.vector.tensor_tensor(
                                out=OT[p][64:128, qsl], in0=OT[p][64:128, qsl],
                                in1=rbc[p][64:128, :],
                                op=mybir.AluOpType.mult,
                            )
                        return go

                    return [s_po(0), s_po(1), s_gather_recip,
                            s_bcast(0), s_bcast(1), s_scale(0), s_scale(1)]

                def emit_pv(po, kt, pts):
                    for p in range(2):
                        base = p * 193
                        nc.tensor.matmul(
                            po[p][0:65, 0:512],
                            vha[kt][:, base:base + 65],
                            pts[p][:, 0:512],
                            start=(kt == 0), stop=(kt == KT - 1),
                        )
                        nc.tensor.matmul(
                            po[p][:, 512:1024],
                            vha[kt][:, base + 65:base + 193],
                            pts[p][:, 512:1024],
                            start=(kt == 0), stop=(kt == KT - 1),
                        )

                pending = []
                for qt in range(QT):
                    po = [pvp.tile([128, 1024], F32, tag="po", name="po",
                                   bufs=2) for _ in range(2)]
                    # PV matmuls trail the scores by one kt so the PE queue
                    # always holds ready score work when a boundary stalls
                    # the PV chain.
                    prev_pv = None
                    for kt in range(KT):
                        mt = singles.tile([128, 512], BF16, tag="mask",
                                          name="mask", bufs=6)
                        nc.sync.dma_start(out=mt, in_=mk_d[kt, qt])
                        m_ap = mt[:, :]
                        mbc = bass.AP(
                            tensor=m_ap.tensor,
                            offset=m_ap.offset,
                            ap=[list(m_ap.ap[0]), [0, 2], list(m_ap.ap[1])],
                        )
                        pts = []
                        for p in range(2):
                            ps = scp.tile([128, 1024], F32, tag="sc", name="ps")
                            for ab in range(2):
                                nc.tensor.matmul(
                                    ps[:, ab * 512:(ab + 1) * 512],
                                    khT[p][ab * 64:(ab + 1) * 64,
                                           kt * 128:(kt + 1) * 128],
                                    qhT[p][ab * 64:(ab + 1) * 64,
                                           qt * 512:(qt + 1) * 512],
                                    start=True,
                                    stop=True,
                                )
                            pt = singles.tile([128, 1024], BF16, tag="pt",
                                              name="pt", bufs=8)
                            nc.scalar.activation(
                                out=pt, in_=ps,
                                func=mybir.ActivationFunctionType.Exp,
                                scale=float(SCALE),
                            )
                            nc.vector.tensor_tensor(
                                out=pt, in0=pt, in1=mbc,
                                op=mybir.AluOpType.mult,
                            )
                            pts.append(pt)
                        if prev_pv is not None:
                            emit_pv(po, *prev_pv)
                        prev_pv = (kt, pts)
                        if pending:
                            pending.pop(0)()
                    emit_pv(po, *prev_pv)
                    while pending:
                        pending.pop(0)()
                    pending = make_norm_tail(qt, po)

                # ---- output projection (reuses scp rotation; PE stays hot).
                # qt3's norm tail interleaves into the half0 emissions so the
                # oproj psum slots are requested before the rps broadcasts.
                for half in range(2):
                    for ot in range(8):
                        idx = half * 8 + ot
                        if idx % 2 == 0:
                            ps = scp.tile([128, 1024], F32, tag="sc", name="psy")
                        else:
                            ps = pvp.tile([128, 1024], F32, tag="po", name="psy")
                        for p in range(2):
                            for n in range(2):
                                nc.tensor.matmul(
                                    ps[:, n * 512:(n + 1) * 512],
                                    wo_sb[p][:, ot * 128:(ot + 1) * 128],
                                    OT[p][:, (half * 2 + n) * 512:
                                          (half * 2 + n + 1) * 512],
                                    start=(p == 0),
                                    stop=(p == 1),
                                )
                        yt = singles.tile([128, 1024], BF16, tag="yt",
                                          name="yt", bufs=4)
                        if idx % 2 == 0:
                            nc.scalar.copy(out=yt, in_=ps)
                            nc.sync.dma_start(
                                out=yt_d[ot][:, half * 1024:(half + 1) * 1024],
                                in_=yt)
                        else:
                            nc.vector.tensor_copy(out=yt, in_=ps)
                            nc.scalar.dma_start(
                                out=yt_d[ot][:, half * 1024:(half + 1) * 1024],
                                in_=yt)
                        if pending:
                            pending.pop(0)()
    nc.compile()
    return nc


_NC_CACHE = None


def get_nc():
    global _NC_CACHE
    if _NC_CACHE is None:
        _NC_CACHE = build_nc()
    return _NC_CACHE


def _tile_ct(xT):
    # [1024, N] -> [128, CT, N]  (c-block-major partition layout)
    n = xT.shape[1]
    return np.ascontiguousarray(xT.reshape(CT, 128, n).transpose(1, 0, 2))


def prep_in_maps(q, k, v, mask, Wq, bq, Wk, bk, Wv, bv, Wo, bo):
    q = np.asarray(q, np.float32)
    k = np.asarray(k, np.float32)
    v = np.asarray(v, np.float32)
    mask = np.asarray(mask)
    WqT = np.asarray(Wq, np.float32).T
    WkT = np.asarray(Wk, np.float32).T
    WvT = np.asarray(Wv, np.float32).T
    WoT = np.asarray(Wo, np.float32).T
    bq = np.asarray(bq, np.float32)
    bk = np.asarray(bk, np.float32)
    bv = np.asarray(bv, np.float32)

    xT = {}
    keepT = {}
    for b in range(B):
        xT[b] = (
            _tile_ct(np.ascontiguousarray(q[b].T)).astype(NP_F8),
            _tile_ct(np.ascontiguousarray(k[b].T)).astype(NP_F8),
            _tile_ct(np.ascontiguousarray(v[b].T)).astype(NP_BF16),
        )
        mt = np.ascontiguousarray((~mask[b, 0]).T.astype(np.float32)).astype(NP_BF16)
        keepT[b] = np.ascontiguousarray(
            mt.reshape(KT, 128, QT, 512).transpose(0, 2, 1, 3))

    sel = np.zeros((2, 128), np.float32)
    sel[0, 64:128] = 1.0  # row0 <- partition-32 (B) sums -> dims 64:128
    sel[1, 0:64] = 1.0    # row1 <- partition-64 (A) sums -> dims 0:64
    sel = sel.astype(NP_BF16)

    in_maps = []
    for c in range(N_CORES):
        b = c // 4
        ho = c % 4
        dsl = slice(ho * 256, ho * 256 + 256)
        xq, xk, xv = xT[b]
        in_maps.append({
            "xq": xq,
            "xk": xk,
            "xv": xv,
            "wq": _tile_ct(np.ascontiguousarray(WqT[:, dsl])).astype(NP_F8),
            "wk": _tile_ct(np.ascontiguousarray(WkT[:, dsl])).astype(NP_F8),
            "wv": _tile_ct(np.ascontiguousarray(WvT[:, dsl])).astype(NP_BF16),
            "wo": np.ascontiguousarray(WoT[dsl, :]).astype(NP_BF16).reshape(2, 128, 1024),
            "bcom": np.concatenate([
                bq[dsl].reshape(2, 128).T,
                bk[dsl].reshape(2, 128).T,
                np.broadcast_to(bv[dsl], (128, 256)),
            ], axis=1).astype(np.float32),
            "mk": keepT[b],
            "selc": sel,
        })
    return in_maps


def gather_output(results, bo):
    bo = np.asarray(bo, np.float32)
    y = np.zeros((B, S, DIM), np.float32)
    for c in range(N_CORES):
        yt = np.asarray(results[c]["yt"], np.float32)  # [8, 128, 2048]
        yT = yt.reshape(DIM, S)
        y[c // 4] += yT.T
    y += bo[None, None, :]
    return y


def kernel(**inputs):
    nc = get_nc()
    in_maps = prep_in_maps(**{k_: inputs[k_] for k_ in (
        "q", "k", "v", "mask", "Wq", "bq", "Wk", "bk", "Wv", "bv", "Wo", "bo")})
    res = bass_utils.run_bass_kernel_spmd(nc, in_maps, list(range(N_CORES)))
    return gather_output(res.results, inputs["bo"])


# revision 35
# speedup vs baseline: 1.2541x; 1.0196x over previous
"""MultiHeadAttention Trainium2 Bass kernel (8-core SPMD), v2.

Problem: B=2, S=2048, DIM=1024, H=16 heads (dh=64), fp32 reference.
Sharding: core c handles batch b = c//4 and 4 heads ho = 4*(c%4)..+4.

v2 changes vs v1 (332us -> target ~180us):
- q/k path in fp8e4m3 (x AND W, host-validated rel-err 3.6e-3 vs 2e-2
  budget); v path stays bf16 (fp8 there costs 1.3e-2).
- q/k projections use MatmulPerfMode.DoubleRow (fp8 K=256 contraction).
- x / W tensors shipped as single [128, CT, *] tiles -> one large DMA
  each with 4-32KB per-partition contiguous runs (~350+ GB/s vs ~200).
- Per-qt pipelined softmax normalization: sums -> recip -> selector-
  matmul partition-broadcast -> in-place OT scale, all hidden under the
  next qt's attention (replaces a 20us end-of-kernel stall + DRAM
  round-trips).
- Output projection PSUM reuses the scores pool rotation so the PE
  never idles between attention and oproj (HAM stays warm); PSUM->SBUF
  output copies alternate ACT/DVE.
- Attention phase is ACT(exp)-bound (~1us per [128,1024] exp); PE/DVE
  loads are kept strictly below that (~0.86us / ~0.85us per half-tile).
"""

import os
import sys

sys.path.insert(0, "/opt/trn_rl_repo")
os.environ.setdefault("MYCRO_LOCAL_CACHE", "1")

import numpy as np

import concourse.bass as bass
import concourse.bacc as bacc
import concourse.tile as tile
from concourse import mybir
from concourse import bass_utils

F32 = mybir.dt.float32
BF16 = mybir.dt.bfloat16
F8 = mybir.dt.float8e4
NP_BF16 = mybir.dt.np(BF16)
NP_F8 = mybir.dt.np(F8)
DR = mybir.MatmulPerfMode.DoubleRow

B, S, DIM = 2, 2048, 1024
H = 16
DH = 64
SCALE = 1.0 / (DIM ** 0.5)
N_CORES = 8
HPC = 4          # heads per core
QT = S // 512    # 4 q-chunks of 512
KT = S // 128    # 16 k-tiles of 128
CT = DIM // 128  # 8 contraction tiles for projections

# vh_aug per-kt layout (unchanged from v1): per pair p (2 local pairs):
#   A block: [vh_A(64) | ones(1)]                 at cols p*193 + [0, 65)
#   B block: [zeros(32) | ones(1) | zeros(31) | vh_B(64)] at cols p*193 + [65, 193)
VHA_W = 386


def build_nc():
    nc = bacc.Bacc("TRN2", target_bir_lowering=False)

    xq_d = nc.declare_dram_parameter("xq", [128, CT, S], F8, isOutput=False)
    xk_d = nc.declare_dram_parameter("xk", [128, CT, S], F8, isOutput=False)
    xv_d = nc.declare_dram_parameter("xv", [128, CT, S], BF16, isOutput=False)
    wq_d = nc.declare_dram_parameter("wq", [128, CT, 256], F8, isOutput=False)
    wk_d = nc.declare_dram_parameter("wk", [128, CT, 256], F8, isOutput=False)
    wv_d = nc.declare_dram_parameter("wv", [128, CT, 256], BF16, isOutput=False)
    wo_d = nc.declare_dram_parameter("wo", [2, 128, 1024], BF16, isOutput=False)
    # packed biases: cols 0:2 = bq halves, 2:4 = bk halves, 4:260 = bv bcast
    bc_d = nc.declare_dram_parameter("bcom", [128, 260], F32, isOutput=False)
    mk_d = nc.declare_dram_parameter("mk", [KT, QT, 128, 512], BF16, isOutput=False)
    sel_d = nc.declare_dram_parameter("selc", [2, 128], BF16, isOutput=False)
    yt_d = nc.declare_dram_parameter("yt", [8, 128, 2048], BF16, isOutput=True)

    with tile.TileContext(nc) as tc:
        with tc.tile_pool(name="persist", bufs=1) as singles:
            # ---- small operands first on the DMA queue (packed biases) ----
            bc_sb = singles.tile([128, 260], F32, tag="bcom", name="bcom")
            nc.sync.dma_start(out=bc_sb, in_=bc_d[:, :])
            bq_sb = [bc_sb[:, m:m + 1] for m in range(2)]
            bk_sb = [bc_sb[:, 2 + m:3 + m] for m in range(2)]
            bvb_sb = bc_sb[:, 4:260]

            wk_sb = singles.tile([128, CT, 256], F8, tag="wk", name="wk")
            nc.sync.dma_start(out=wk_sb, in_=wk_d[:, :, :])
            wq_sb = singles.tile([128, CT, 256], F8, tag="wq", name="wq")
            nc.sync.dma_start(out=wq_sb, in_=wq_d[:, :, :])
            wv_sb = singles.tile([128, CT, 256], BF16, tag="wv", name="wv")
            nc.sync.dma_start(out=wv_sb, in_=wv_d[:, :, :])
            wo_sb = []
            for m in range(2):
                t = singles.tile([128, 1024], BF16, tag=f"wo{m}", name=f"wo{m}")
                nc.sync.dma_start(out=t, in_=wo_d[m])
                wo_sb.append(t)

            # ---- bulk x in consumption order: k, v, q ----
            xk_sb = singles.tile([128, CT, S], F8, tag="xk", name="xk")
            nc.sync.dma_start(out=xk_sb, in_=xk_d[:, :, :])
            xv_sb = singles.tile([128, CT, S], BF16, tag="xv", name="xv")
            nc.sync.dma_start(out=xv_sb, in_=xv_d[:, :, :])
            xq_sb = singles.tile([128, CT, S], F8, tag="xq", name="xq")
            nc.sync.dma_start(out=xq_sb, in_=xq_d[:, :, :])

            # ---- persistent intermediates ----
            qhT = [singles.tile([128, S], BF16, tag=f"qhT{m}", name=f"qhT{m}")
                   for m in range(2)]
            khT = [singles.tile([128, S], BF16, tag=f"khT{m}", name=f"khT{m}")
                   for m in range(2)]
            OT = [singles.tile([128, S], BF16, tag=f"OT{m}", name=f"OT{m}")
                  for m in range(2)]
            vha = [singles.tile([128, VHA_W], BF16, tag=f"vha{kt}",
                                name=f"vha{kt}") for kt in range(KT)]
            sums_st = singles.tile([128, 2, 512], F32, tag="sums_st")
            sel_sb = singles.tile([2, 128], BF16, tag="sel")

            # warm tile memset FIRST: the PE warmup gates on it, and the
            # vha/sums memsets behind it cost ~15us of gpsimd time.
            warm = singles.tile([128, 512], BF16, tag="warm")
            nc.gpsimd.memset(warm[:, :], 0.0)
            nc.gpsimd.memset(sums_st[:, :, :], 1.0)
            for kt in range(KT):
                for p in range(2):
                    base = p * 193
                    nc.gpsimd.memset(vha[kt][:, base + 64:base + 65], 1.0)
                    nc.gpsimd.memset(vha[kt][:, base + 97:base + 98], 1.0)
                    nc.gpsimd.memset(vha[kt][:, base + 65:base + 97], 0.0)
                    nc.gpsimd.memset(vha[kt][:, base + 98:base + 129], 0.0)
            # selector: row0 (gathered from partition 32) = B sums -> dims 64:128
            #           row1 (partition 64) = A sums -> dims 0:64
            nc.sync.dma_start(out=sel_sb, in_=sel_d[:, :])

            # ---- projections ----
            with tc.tile_pool(name="pjp", bufs=2, space="PSUM") as pj:
                # PE warmup to open the HAM clock gate while DMAs land
                wps = pj.tile([128, 512], F32, tag="pwarm", name="wps")
                for i in range(48):
                    nc.tensor.matmul(
                        wps, warm[:, 0:128], warm[:, :],
                        start=True, stop=True)

                def qk_proj(w_sb, x_sb, b_sb, dst, m):
                    # weight (c-pair, m-half) stays stationary across the 4
                    # n-chunks -> DoubleRow LDWEIGHTS amortized 4x.
                    pss = [pj.tile([128, 512], F32, tag=f"pqk{n}",
                                   name=f"psqk{n}", bufs=1) for n in range(4)]
                    for ci in range(4):
                        for n in range(4):
                            nc.tensor.matmul(
                                pss[n],
                                w_sb[:, 2 * ci:2 * ci + 2, m * 128:(m + 1) * 128],
                                x_sb[:, 2 * ci:2 * ci + 2, n * 512:(n + 1) * 512],
                                start=(ci == 0),
                                stop=(ci == 3),
                                perf_mode=DR,
                            )
                    bb = b_sb[m][:, 0:1]
                    bb_bc = bass.AP(
                        tensor=bb.tensor, offset=bb.offset,
                        ap=[list(bb.ap[0]), [0, 512]])
                    for n in range(4):
                        nc.vector.tensor_tensor(
                            out=dst[m][:, n * 512:(n + 1) * 512],
                            in0=pss[n], in1=bb_bc,
                            op=mybir.AluOpType.add,
                        )

                qk_proj(wk_sb, xk_sb, bk_sb, khT, 0)
                qk_proj(wk_sb, xk_sb, bk_sb, khT, 1)
                # v-projection (bf16)
                for kt in range(KT):
                    ps = pj.tile([128, 256], F32, tag="pv", name="psv")
                    for c in range(CT):
                        nc.tensor.matmul(
                            ps,
                            xv_sb[:, c, kt * 128:(kt + 1) * 128],
                            wv_sb[:, c, :],
                            start=(c == 0),
                            stop=(c == CT - 1),
                        )
                    for h in range(HPC):
                        p, is_b = h // 2, h % 2
                        col = p * 193 + (129 if is_b else 0)
                        nc.vector.tensor_tensor(
                            out=vha[kt][:, col:col + 64],
                            in0=ps[:, h * 64:(h + 1) * 64],
                            in1=bvb_sb[:, h * 64:(h + 1) * 64],
                            op=mybir.AluOpType.add,
                        )
                qk_proj(wq_sb, xq_sb, bq_sb, qhT, 0)
                qk_proj(wq_sb, xq_sb, bq_sb, qhT, 1)
                # (m is the head-pair index; both m needed before attention)

            # ---- attention + pipelined normalization + oproj ----
            with tc.tile_pool(name="scp", bufs=2, space="PSUM") as scp, \
                 tc.tile_pool(name="pvp", bufs=2, space="PSUM") as pvp:

                def make_norm_tail(qt, po):
                    """All qt-end work, split into ~1.4us steps interleaved
                    between the next qt's kt iterations (or oproj tiles).
                    po[p] frees after steps 2p and 2p+1."""
                    qsl = slice(qt * 512, (qt + 1) * 512)
                    rec_in = singles.tile([2, 2, 512], F32, tag="rec_in",
                                          name="rec_in", bufs=2)
                    rec_f = singles.tile([2, 2, 512], F32, tag="rec_f",
                                         name="rec_f", bufs=2)
                    rec_bf = singles.tile([2, 2, 512], BF16, tag="rec_bf",
                                          name="rec_bf", bufs=2)
                    rbc = [None, None]

                    def s_po(p):
                        # OT casts on DVE, sum staging on ACT (parallel);
                        # po[p] frees when all four complete.
                        def go():
                            nc.vector.tensor_copy(
                                out=OT[p][0:64, qsl], in_=po[p][0:64, 0:512])
                            nc.scalar.copy(
                                out=sums_st[64:65, p, :], in_=po[p][64:65, 0:512])
                            nc.vector.tensor_copy(
                                out=OT[p][64:128, qsl],
                                in_=po[p][64:128, 512:1024])
                            nc.scalar.copy(
                                out=sums_st[32:33, p, :],
                                in_=po[p][32:33, 512:1024])
                        return go

                    def s_gather_recip():
                        s32 = sums_st[32:33, :, :]
                        s64 = sums_st[64:65, :, :]
                        src = bass.AP(
                            tensor=s32.tensor, offset=s32.offset,
                            ap=[[s64.offset - s32.offset, 2],
                                list(s32.ap[1]), list(s32.ap[2])])
                        nc.sync.dma_start(out=rec_in, in_=src)
                        nc.vector.reciprocal_approx_fast(out=rec_f, in_=rec_in)
                        nc.vector.tensor_copy(out=rec_bf, in_=rec_f)

                    def s_bcast(p):
                        def go():
                            rps = scp.tile([128, 1024], F32, tag="sc", name="rps")
                            nc.tensor.matmul(
                                rps[:, 0:512], sel_sb, rec_bf[:, p, :],
                                start=True, stop=True)
                            rb = singles.tile([128, 512], BF16, tag=f"rbc{p}",
                                              name=f"rbc{p}", bufs=2)
                            nc.vector.tensor_copy(out=rb, in_=rps[:, 0:512])
                            rbc[p] = rb
                        return go

                    def s_scale(p):
                        def go():
                            nc.vector.tensor_tensor(
                                out=OT[p][0:64, qsl], in0=OT[p][0:64, qsl],
                                in1=rbc[p][0:64, :],
                                op=mybir.AluOpType.mult,
                            )
                            nc.vector.tensor_tensor(
                                out=OT[p][64:128, qsl], in0=OT[p][64:128, qsl],
                                in1=rbc[p][64:128, :],
                                op=mybir.AluOpType.mult,
                            )
                        return go

                    return [s_po(0), s_po(1), s_gather_recip,
                            s_bcast(0), s_bcast(1), s_scale(0), s_scale(1)]

                def emit_pv(po, kt, pts):
                    for p in range(2):
                        base = p * 193
                        nc.tensor.matmul(
                            po[p][0:65, 0:512],
                            vha[kt][:, base:base + 65],
                            pts[p][:, 0:512],
                            start=(kt == 0), stop=(kt == KT - 1),
                        )
                        nc.tensor.matmul(
                            po[p][:, 512:1024],
                            vha[kt][:, base + 65:base + 193],
                            pts[p][:, 512:1024],
                            start=(kt == 0), stop=(kt == KT - 1),
                        )

                pending = []
                for qt in range(QT):
                    po = [pvp.tile([128, 1024], F32, tag="po", name="po",
                                   bufs=2) for _ in range(2)]
                    # PV matmuls trail the scores by one kt so the PE queue
                    # always holds ready score work when a boundary stalls
                    # the PV chain.
                    prev_pv = None
                    for kt in range(KT):
                        mt = singles.tile([128, 512], BF16, tag="mask",
                                          name="mask", bufs=6)
                        nc.sync.dma_start(out=mt, in_=mk_d[kt, qt])
                        m_ap = mt[:, :]
                        mbc = bass.AP(
                            tensor=m_ap.tensor,
                            offset=m_ap.offset,
                            ap=[list(m_ap.ap[0]), [0, 2], list(m_ap.ap[1])],
                        )
                        pts = []
                        for p in range(2):
                            ps = scp.tile([128, 1024], F32, tag="sc", name="ps")
                            for ab in range(2):
                                nc.tensor.matmul(
                                    ps[:, ab * 512:(ab + 1) * 512],
                                    khT[p][ab * 64:(ab + 1) * 64,
                                           kt * 128:(kt + 1) * 128],
                                    qhT[p][ab * 64:(ab + 1) * 64,
                                           qt * 512:(qt + 1) * 512],
                                    start=True,
                                    stop=True,
                                )
                            pt = singles.tile([128, 1024], BF16, tag="pt",
                                              name="pt", bufs=8)
                            nc.scalar.activation(
                                out=pt, in_=ps,
                                func=mybir.ActivationFunctionType.Exp,
                                scale=float(SCALE),
                            )
                            nc.vector.tensor_tensor(
                                out=pt, in0=pt, in1=mbc,
                                op=mybir.AluOpType.mult,
                            )
                            pts.append(pt)
                        if prev_pv is not None:
                            emit_pv(po, *prev_pv)
                        prev_pv = (kt, pts)
                        # sparse pops: po-frees right away, the recip/bcast/
                        # scale chain spread out so it never blocks mask TTs
                        if pending and kt in (0, 1, 3, 5, 7, 9, 11, 13):
                            pending.pop(0)()
                    emit_pv(po, *prev_pv)
                    while pending:
                        pending.pop(0)()
                    pending = make_norm_tail(qt, po)

                # ---- output projection (reuses scp rotation; PE stays hot).
                # qt3's norm tail interleaves into the half0 emissions so the
                # oproj psum slots are requested before the rps broadcasts.
                for half in range(2):
                    for ot in range(8):
                        idx = half * 8 + ot
                        if idx % 2 == 0:
                            ps = scp.tile([128, 1024], F32, tag="sc", name="psy")
                        else:
                            ps = pvp.tile([128, 1024], F32, tag="po", name="psy")
                        for p in range(2):
                            for n in range(2):
                                nc.tensor.matmul(
                                    ps[:, n * 512:(n + 1) * 512],
                                    wo_sb[p][:, ot * 128:(ot + 1) * 128],
                                    OT[p][:, (half * 2 + n) * 512:
                                          (half * 2 + n + 1) * 512],
                                    start=(p == 0),
                                    stop=(p == 1),
                                )
                        yt = singles.tile([128, 1024], BF16, tag="yt",
                                          name="yt", bufs=4)
                        if idx % 2 == 0:
                            nc.scalar.copy(out=yt, in_=ps)
                            nc.sync.dma_start(
                                out=yt_d[ot][:, half * 1024:(half + 1) * 1024],
                                in_=yt)
                        else:
                            nc.vector.tensor_copy(out=yt, in_=ps)
                            nc.scalar.dma_start(
                                out=yt_d[ot][:, half * 1024:(half + 1) * 1024],
                                in_=yt)
                        if pending:
                            pending.pop(0)()
    nc.compile()
    return nc


_NC_CACHE = None


def get_nc():
    global _NC_CACHE
    if _NC_CACHE is None:
        _NC_CACHE = build_nc()
    return _NC_CACHE


def _tile_ct(xT):
    # [1024, N] -> [128, CT, N]  (c-block-major partition layout)
    n = xT.shape[1]
    return np.ascontiguousarray(xT.reshape(CT, 128, n).transpose(1, 0, 2))


def prep_in_maps(q, k, v, mask, Wq, bq, Wk, bk, Wv, bv, Wo, bo):
    q = np.asarray(q, np.float32)
    k = np.asarray(k, np.float32)
    v = np.asarray(v, np.float32)
    mask = np.asarray(mask)
    WqT = np.asarray(Wq, np.float32).T
    WkT = np.asarray(Wk, np.float32).T
    WvT = np.asarray(Wv, np.float32).T
    WoT = np.asarray(Wo, np.float32).T
    bq = np.asarray(bq, np.float32)
    bk = np.asarray(bk, np.float32)
    bv = np.asarray(bv, np.float32)

    xT = {}
    keepT = {}
    for b in range(B):
        xT[b] = (
            _tile_ct(np.ascontiguousarray(q[b].T)).astype(NP_F8),
            _tile_ct(np.ascontiguousarray(k[b].T)).astype(NP_F8),
            _tile_ct(np.ascontiguousarray(v[b].T)).astype(NP_BF16),
        )
        mt = np.ascontiguousarray((~mask[b, 0]).T.astype(np.float32)).astype(NP_BF16)
        keepT[b] = np.ascontiguousarray(
            mt.reshape(KT, 128, QT, 512).transpose(0, 2, 1, 3))

    sel = np.zeros((2, 128), np.float32)
    sel[0, 64:128] = 1.0  # row0 <- partition-32 (B) sums -> dims 64:128
    sel[1, 0:64] = 1.0    # row1 <- partition-64 (A) sums -> dims 0:64
    sel = sel.astype(NP_BF16)

    in_maps = []
    for c in range(N_CORES):
        b = c // 4
        ho = c % 4
        dsl = slice(ho * 256, ho * 256 + 256)
        xq, xk, xv = xT[b]
        in_maps.append({
            "xq": xq,
            "xk": xk,
            "xv": xv,
            "wq": _tile_ct(np.ascontiguousarray(WqT[:, dsl])).astype(NP_F8),
            "wk": _tile_ct(np.ascontiguousarray(WkT[:, dsl])).astype(NP_F8),
            "wv": _tile_ct(np.ascontiguousarray(WvT[:, dsl])).astype(NP_BF16),
            "wo": np.ascontiguousarray(WoT[dsl, :]).astype(NP_BF16).reshape(2, 128, 1024),
            "bcom": np.concatenate([
                bq[dsl].reshape(2, 128).T,
                bk[dsl].reshape(2, 128).T,
                np.broadcast_to(bv[dsl], (128, 256)),
            ], axis=1).astype(np.float32),
            "mk": keepT[b],
            "selc": sel,
        })
    return in_maps


def gather_output(results, bo):
    bo = np.asarray(bo, np.float32)
    y = np.zeros((B, S, DIM), np.float32)
    for c in range(N_CORES):
        yt = np.asarray(results[c]["yt"], np.float32)  # [8, 128, 2048]
        yT = yt.reshape(DIM, S)
        y[c // 4] += yT.T
    y += bo[None, None, :]
    return y


def kernel(**inputs):
    nc = get_nc()
    in_maps = prep_in_maps(**{k_: inputs[k_] for k_ in (
        "q", "k", "v", "mask", "Wq", "bq", "Wk", "bk", "Wv", "bv", "Wo", "bo")})
    res = bass_utils.run_bass_kernel_spmd(nc, in_maps, list(range(N_CORES)))
    return gather_output(res.results, inputs["bo"])
